# revision 1
# baseline (speedup 1.0000x reference)
"""BiLevelRoutingAttention Trainium2 kernel (8-core SPMD).

Sharding: core r handles batch b = r//4 and windows w in [ (r%4)*8, (r%4)*8+8 ).
Routing (region top-k) is computed on host via linearity of the mean:
    q_region = mean_{t,s}(xw) @ Wq + bq  (exact up to fp reassociation).
Spikes are binary -> all attention arithmetic is exact integer math in fp16
(counts <= 2048 are exactly representable). Cross-window kv sums need R
(per-region k^T v) from sibling cores -> AllGather over each batch's 4-core
group.

QKV matmul precision (KQKV env): "f16x3" (default; x,w split into fp16 hi+lo,
3 passes, ~1e-4 rel err), "float32r" (1 pass, ~1e-3), "float32" (exact).
Proj precision (KPJ): "float32r" (default) or "float32".
"""

import numpy as np
import ml_dtypes
import os as _os

# ---- problem constants (hardcoded per contract) ----
T, B, Lt, Lh, Lw, C = 4, 2, 8, 32, 32, 256
WT, WH, WW = 2, 4, 4
W = WT * WH * WW            # 32 windows
GT, GH, GW = Lt // WT, Lh // WH, Lw // WW
S = GT * GH * GW            # 256 tokens per window
H, D = 8, C // 8            # 8 heads, 32 dim
TOPK = 8
SCALE = float(D) ** -0.5
NCORES = 8
NW = 8                      # windows per core
NTOK = T * S                # 1024 token-instances per window

QKV_MODE = _os.environ.get("KQKV", "f16x3")     # f16x3 | float32r | float32
PJ_DT_NAME = _os.environ.get("KPJ", "float32r")  # float32r | float32
# NOTE: DVE tensor_scalar is_ge with fp16 output is numerically wrong on HW
# (2.6e-3 rel err vs 3.1e-6); gpsimd is exact. Keep spikes on gpsimd.
GE_ENG = _os.environ.get("KGE", "gpsimd")

_prog_cache = {}


def _dedup_ldweights(nc, mybir):
    """Drop an InstLdweights whose stationary operand is byte-identical to the
    immediately preceding PE Ldweights with only Matmults in between (the PE
    weight slot still holds the same data). Waits/updates are folded into the
    next PE instruction."""
    ndrop = 0
    for bb in nc.main_func.blocks:
        new_list = []
        last_sig = None
        pending = None   # (waits, updates) from a dropped ldw
        for ins in bb.instructions:
            tn = type(ins).__name__
            if ins.engine != mybir.EngineType.PE:
                new_list.append(ins)
                continue
            if pending is not None and tn in ("InstLdweights", "InstMatmult"):
                si = ins.sync_info
                w = list(si.on_wait) if si and si.on_wait else []
                u = list(si.on_update) if si and si.on_update else []
                ins.sync_info = mybir.SyncInfo(on_wait=pending[0] + w,
                                               on_update=pending[1] + u)
                pending = None
            if tn == "InstLdweights":
                try:
                    ap = ins.ins[0]
                    sig = repr(ap)
                except Exception:
                    sig = None
                if sig is not None and sig == last_sig:
                    si = ins.sync_info
                    w = list(si.on_wait) if si and si.on_wait else []
                    u = list(si.on_update) if si and si.on_update else []
                    pending = (w, u)
                    ndrop += 1
                    continue
                last_sig = sig
            elif tn != "InstMatmult":
                last_sig = None
            new_list.append(ins)
        assert pending is None or not (pending[0] or pending[1])
        bb.instructions[:] = new_list
    return ndrop


def _split_sync_waits(nc, mybir, maxw=1):
    """walrus in this container rejects >1 sync wait per instruction; split
    excess waits onto NoOp instructions inserted just before."""
    for bb in nc.main_func.blocks:
        new_list = []
        for ins in bb.instructions:
            si = ins.sync_info
            waits = list(si.on_wait) if si is not None and si.on_wait else []
            if len(waits) > maxw:
                extra = waits[:-maxw]
                keep = waits[-maxw:]
                idx = 0
                while extra:
                    chunk, extra = extra[:maxw], extra[maxw:]
                    nop = mybir.InstNoOp(name=f"{ins.name}-wsplit{idx}", ins=[], outs=[])
                    nop.engine = ins.engine
                    nop.sync_info = mybir.SyncInfo(on_wait=chunk, on_update=[])
                    new_list.append(nop)
                    idx += 1
                ins.sync_info = mybir.SyncInfo(
                    on_wait=keep,
                    on_update=list(si.on_update) if si.on_update else [],
                )
            new_list.append(ins)
        bb.instructions[:] = new_list


def _build_program(routing_idx, sim_mode=False):
    """routing_idx: [B, W, TOPK] int array (host-computed). Returns nc.
    sim_mode: no collective / no If-chain (single-core TimelineSim)."""
    import contextlib
    import concourse.bass as bass
    import concourse.mybir as mybir
    import concourse.tile as tile

    f32 = mybir.dt.float32
    f16 = mybir.dt.float16
    bf16 = mybir.dt.bfloat16
    pj_dt = getattr(mybir.dt, PJ_DT_NAME)
    split3 = QKV_MODE == "f16x3"
    qkv_dt = f16 if split3 else getattr(mybir.dt, QKV_MODE)
    NSP = 2 if split3 else 1      # of x operand copies (hi, lo)
    ALU = mybir.AluOpType
    ACT = mybir.ActivationFunctionType
    from concourse.dve_ops import TENSOR_MASK as DVE_TENSOR_MASK

    nc = bass.Bass(num_devices=NCORES)
    ge_eng = getattr(nc, GE_ENG)

    # ---- I/O ----
    # x feature-major: [wi, split, kc, c(128), (t,s)]
    x_in = nc.dram_tensor("x_in", [NW, NSP, 2, 128, NTOK], qkv_dt, kind="ExternalInput")
    # w layouts [split, kc, c(128), f]
    wkv_in = nc.dram_tensor("wkv_in", [NSP, 2, 128, 512], qkv_dt, kind="ExternalInput")
    wq_in = nc.dram_tensor("wq_in", [NSP, 2, 128, 256], qkv_dt, kind="ExternalInput")
    # bias rows (pre-halved), bf16 hi+lo exact-ish splits
    bkv_b_in = nc.dram_tensor("bkv_b_in", [2, 512], bf16, kind="ExternalInput")
    bq_b_in = nc.dram_tensor("bq_b_in", [2, 256], bf16, kind="ExternalInput")
    wp_in = nc.dram_tensor("wp_in", [2, 128, 256], pj_dt, kind="ExternalInput")
    bp_in = nc.dram_tensor("bp_in", [2, 128, 1], f32, kind="ExternalInput")
    out_d = nc.dram_tensor("out_d", [NW, 2, 128, NTOK], f32, kind="ExternalOutput")

    with tile.TileContext(nc) as tc:
        with (
            tc.tile_pool(name="const", bufs=1) as constp,
            tc.tile_pool(name="xin", bufs=3) as xin_p,
            tc.tile_pool(name="hbuf", bufs=3) as h_p,
            tc.tile_pool(name="skv", bufs=2) as skv_p,
            tc.tile_pool(name="state", bufs=2) as st_p,
            tc.tile_pool(name="persist", bufs=1) as pers_p,
            tc.tile_pool(name="attn", bufs=2) as attn_p,
            tc.tile_pool(name="outs", bufs=2) as out_p,
            tc.tile_pool(name="dram", bufs=1, space="DRAM") as dram_p,
        ):
            # ---- constants / weights ----
            wkv_sb = constp.tile([128, NSP * 2 * 512], qkv_dt)
            wq_sb = constp.tile([128, NSP * 2 * 256], qkv_dt)
            for sp in range(NSP):
                for kc in range(2):
                    nc.sync.dma_start(
                        wkv_sb[:, (sp * 2 + kc) * 512:(sp * 2 + kc + 1) * 512],
                        wkv_in[sp, kc])
                    nc.sync.dma_start(
                        wq_sb[:, (sp * 2 + kc) * 256:(sp * 2 + kc + 1) * 256],
                        wq_in[sp, kc])
            wp_sb = constp.tile([128, 2 * 256], pj_dt)
            for kc in range(2):
                nc.sync.dma_start(wp_sb[:, kc * 256:(kc + 1) * 256], wp_in[kc])
            bp_sb = constp.tile([128, 2], f32)
            for ftc in range(2):
                nc.sync.dma_start(bp_sb[:, ftc:ftc + 1], bp_in[ftc])
            bkv_b = constp.tile([2, 512], bf16)     # rows: (hi, lo)
            bq_b = constp.tile([2, 256], bf16)
            nc.sync.dma_start(bkv_b[:], bkv_b_in[:])
            nc.sync.dma_start(bq_b[:], bq_b_in[:])
            ones128 = constp.tile([2, 128], bf16)
            nc.vector.memset(ones128[:], 1.0)
            ones512 = constp.tile([2, 512], bf16)
            nc.vector.memset(ones512[:], 1.0)

            # persistent across phases
            r_loc = pers_p.tile([128, 2048], f16)          # local R, (slab,t,wi,e)
            r_all = pers_p.tile([128, 8192], f16)          # gathered R, (slab,t,j,e)
            kv_w = [pers_p.tile([128, 256], f16, name=f"kvw{i}") for i in range(NW)]

            # ================= phase 1: per-window qkv + LIF + R =================
            ph1 = contextlib.ExitStack()
            ps_a = ph1.enter_context(tc.tile_pool(name="psA", bufs=2, space="PSUM"))
            ps_b = ph1.enter_context(tc.tile_pool(name="psB", bufs=1, space="PSUM"))
            ps_r = ph1.enter_context(tc.tile_pool(name="psr", bufs=1, space="PSUM"))
            for wi in range(NW):
                x_sb = xin_p.tile([128, NSP * 2 * NTOK], qkv_dt, tag="xsb")
                for sp in range(NSP):
                    for kc in range(2):
                        nc.sync.dma_start(
                            x_sb[:, (sp * 2 + kc) * NTOK:(sp * 2 + kc + 1) * NTOK],
                            x_in[wi, sp, kc])

                def xsl(sp, kc, lo, hi):
                    return x_sb[:, (sp * 2 + kc) * NTOK + lo:(sp * 2 + kc) * NTOK + hi]

                skv = skv_p.tile([128, 8 * 512], f16, tag="skv")

                # ---- pass A matmuls + LIF (k,v; token-major), both halves ----
                vkv = st_p.tile([128, 1024], f32, tag="vkv")
                lt = st_p.tile([128, 1024], f32, tag="ltkv")
                for t in range(T):
                    ps = ps_a.tile([128, 1024], f32, tag="psa")
                    for sh in range(2):
                        st = t * 2 + sh
                        reg = ps[:, sh * 512:(sh + 1) * 512]
                        first = True
                        if split3:
                            for kc in range(2):
                                for (sx, sw) in ((0, 0), (0, 1), (1, 0)):
                                    nc.tensor.matmul(
                                        reg,
                                        lhsT=xsl(sx, kc, st * 128, (st + 1) * 128),
                                        rhs=wkv_sb[:, (sw * 2 + kc) * 512:(sw * 2 + kc + 1) * 512],
                                        start=first, stop=False)
                                    first = False
                        else:
                            for kc in range(2):
                                nc.tensor.matmul(
                                    reg,
                                    lhsT=xsl(0, kc, st * 128, (st + 1) * 128),
                                    rhs=wkv_sb[:, kc * 512:(kc + 1) * 512],
                                    start=first, stop=False)
                                first = False
                        nc.tensor.matmul(reg, lhsT=ones128[:], rhs=bkv_b[:],
                                         start=False, stop=True)
                    # LIF step on [128,1024]; state vkv = 2*V ("W")
                    sksl = skv[:, (t * 2) * 512:(t * 2 + 2) * 512]
                    hsb = h_p.tile([128, 1024], f32, tag="hkv")
                    if t == 0:
                        nc.scalar.activation(hsb[:], ps[:], ACT.Copy, bias=0.0, scale=1.0)
                    else:
                        nc.vector.tensor_tensor(hsb[:], ps[:], vkv[:], op=ALU.add)
                    ge_eng.tensor_scalar(sksl, hsb[:], 1.0, None, ALU.is_ge)
                    if t < T - 1:
                        nc.vector.tensor_scalar(lt[:], hsb[:], 1.0, 0.5, ALU.is_lt, ALU.mult)
                        nc.vector.tensor_tensor(vkv[:], hsb[:], lt[:], op=ALU.mult)

                # ---- R = k^T v per (t, head): [d,e] blocks, col-tiled 4 heads ----
                for t in range(T):
                    for slab in range(2):
                        psr = ps_r.tile([128, 32], f32, tag="psr")
                        for hl in range(4):
                            h = slab * 4 + hl
                            for sh in range(2):
                                st = t * 2 + sh
                                nc.tensor.matmul(
                                    psr[32 * hl:32 * (hl + 1), :],
                                    lhsT=skv[:, st * 512 + h * 32: st * 512 + (h + 1) * 32],
                                    rhs=skv[:, st * 512 + 256 + h * 32: st * 512 + 256 + (h + 1) * 32],
                                    start=(sh == 0), stop=(sh == 1),
                                    tile_position=(0, 32 * hl),
                                )
                        nc.scalar.activation(
                            r_loc[:, ((slab * 4 + t) * 8 + wi) * 32:
                                  ((slab * 4 + t) * 8 + wi + 1) * 32],
                            psr[:], ACT.Copy, bias=0.0, scale=1.0)

            ph1.close()
            # ================= phase 2: exchange R, kv sums, attention, proj ======
            ph2 = contextlib.ExitStack()
            ps_at = ph2.enter_context(tc.tile_pool(name="psat", bufs=2, space="PSUM"))
            ps_pj = ph2.enter_context(tc.tile_pool(name="pspj", bufs=2, space="PSUM"))
            rb_in = dram_p.tile([128, 2048], f16)
            rb_out = dram_p.tile([4, 128, 2048], f16)
            nc.sync.dma_start(rb_in[:], r_loc[:])
            if sim_mode:
                for rk in range(4):
                    nc.sync.dma_start(rb_out[rk], rb_in[:])
            else:
                nc.gpsimd.collective_compute(
                    "AllGather",
                    mybir.AluOpType.bypass,
                    replica_groups=[[0, 1, 2, 3], [4, 5, 6, 7]],
                    ins=[rb_in[:].opt()],
                    outs=[rb_out[:].opt()],
                )
            # r_all free layout: (slab2, t4, j32, e32)
            r_all_v = r_all[:].rearrange("p (a t j e) -> p a t j e", a=2, t=4, j=32, e=32)
            for rk in range(4):
                src = rb_out[rk].rearrange("p (a t w e) -> p a t w e", a=2, t=4, w=8, e=32)
                nc.sync.dma_start(r_all_v[:, :, :, rk * 8:(rk + 1) * 8, :], src)

            # kv sums: routed gather baked per core, guarded by If on core id
            pid = None if sim_mode else nc.partition_id()
            for r in range(NCORES):
                if sim_mode and r != 0:
                    continue
                b_of = r // 4
                wg = r % 4
                with (contextlib.nullcontext() if sim_mode else tc.If(pid == r)):
                    for wl in range(NW):
                        wglob = wg * 8 + wl
                        idxs = [int(j) for j in routing_idx[b_of, wglob]]
                        eng = nc.gpsimd if (wl % 4 == 3) else nc.vector
                        dst = kv_w[wl][:]
                        src0 = r_all_v[:, :, :, idxs[0], :]
                        eng.tensor_copy(dst, src0)
                        for j in idxs[1:]:
                            eng.tensor_tensor(
                                dst, dst, r_all_v[:, :, :, j, :], op=ALU.add)

            # ============ segment 2: q matmuls + LIF-q + attention + proj ========
            ps_b2 = ph2.enter_context(tc.tile_pool(name="psB2", bufs=2, space="PSUM"))
            for wi in range(NW):
                x_sb = xin_p.tile([128, NSP * 2 * NTOK], qkv_dt, tag="xsb")
                for sp in range(NSP):
                    for kc in range(2):
                        nc.sync.dma_start(
                            x_sb[:, (sp * 2 + kc) * NTOK:(sp * 2 + kc + 1) * NTOK],
                            x_in[wi, sp, kc])

                def xsl(sp, kc, lo, hi):
                    return x_sb[:, (sp * 2 + kc) * NTOK + lo:(sp * 2 + kc) * NTOK + hi]

                sq_w = skv_p.tile([128, 2048], f16, tag="sqw")
                # ---- pass B matmuls + LIF (q; feature-major), both ftc batched ----
                vq = st_p.tile([128, 512], f32, tag="vq")
                ltq = st_p.tile([128, 512], f32, tag="ltq")
                for nch in range(2):
                    psq = ps_b2.tile([128, 1024], f32, tag="psb2")
                    for ftc in range(2):
                        reg = psq[:, ftc * 512:(ftc + 1) * 512]
                        first = True
                        if split3:
                            for kc in range(2):
                                for (sx, sw) in ((0, 0), (1, 0), (0, 1)):
                                    nc.tensor.matmul(
                                        reg,
                                        lhsT=wq_sb[:, (sw * 2 + kc) * 256 + ftc * 128:
                                                   (sw * 2 + kc) * 256 + (ftc + 1) * 128],
                                        rhs=xsl(sx, kc, nch * 512, (nch + 1) * 512),
                                        start=first, stop=False)
                                    first = False
                        else:
                            for kc in range(2):
                                nc.tensor.matmul(
                                    reg,
                                    lhsT=wq_sb[:, kc * 256 + ftc * 128:
                                               kc * 256 + (ftc + 1) * 128],
                                    rhs=xsl(0, kc, nch * 512, (nch + 1) * 512),
                                    start=first, stop=False)
                                first = False
                        nc.tensor.matmul(reg, lhsT=bq_b[:, ftc * 128:(ftc + 1) * 128],
                                         rhs=ones512[:], start=False, stop=True)
                    psq_v = psq[:].rearrange("p (f tl e) -> p f tl e", f=2, tl=2, e=256)
                    sq_v = sq_w[:].rearrange("p (f t e) -> p f t e", f=2, t=4, e=256)
                    vq_v = vq[:].rearrange("p (f e) -> p f e", f=2, e=256)
                    ltq_v = ltq[:].rearrange("p (f e) -> p f e", f=2, e=256)
                    for tl in range(2):
                        t = nch * 2 + tl
                        X = psq_v[:, :, tl, :]
                        sqsl = sq_v[:, :, t, :]
                        hq = h_p.tile([128, 512], f32, tag="hq")
                        hq_v = hq[:].rearrange("p (f e) -> p f e", f=2, e=256)
                        if t == 0:
                            nc.scalar.activation(hq_v, X, ACT.Copy, bias=0.0, scale=1.0)
                        else:
                            nc.vector.tensor_tensor(hq_v, X, vq_v, op=ALU.add)
                        ge_eng.tensor_scalar(sqsl, hq_v, 1.0, None, ALU.is_ge)
                        if t < T - 1:
                            nc.vector.tensor_scalar(ltq[:], hq[:], 1.0, 0.5, ALU.is_lt, ALU.mult)
                            nc.vector.tensor_tensor(vq[:], hq[:], ltq[:], op=ALU.mult)

                attn = attn_p.tile([128, 2 * NTOK], pj_dt, tag="attn")
                for t in range(T):
                    for slab in range(2):
                        psa = ps_at.tile([128, 256], f32, tag="psat")
                        for hl in range(4):
                            nc.tensor.matmul(
                                psa[32 * hl:32 * (hl + 1), :],
                                lhsT=kv_w[wi][32 * hl:32 * (hl + 1),
                                              (slab * 4 + t) * 32:
                                              (slab * 4 + t + 1) * 32],
                                rhs=sq_w[32 * hl:32 * (hl + 1),
                                         slab * NTOK + t * 256:
                                         slab * NTOK + (t + 1) * 256],
                                start=True, stop=True,
                                tile_position=(32 * hl, 32 * hl),
                            )
                        dst_at = attn[:, slab * NTOK + t * 256: slab * NTOK + (t + 1) * 256]
                        nc.scalar.activation(dst_at, psa[:], ACT.Copy, bias=0.0, scale=1.0)

                outsb = out_p.tile([128, 2 * NTOK], f32, tag="outsb")
                for cft in range(2):
                    for nch in range(2):
                        psp = ps_pj.tile([128, 512], f32, tag="pspj")
                        for kc in range(2):
                            nc.tensor.matmul(
                                psp[:],
                                lhsT=wp_sb[:, kc * 256 + cft * 128: kc * 256 + (cft + 1) * 128],
                                rhs=attn[:, kc * NTOK + nch * 512: kc * NTOK + (nch + 1) * 512],
                                start=(kc == 0), stop=(kc == 1),
                            )
                        nc.scalar.activation(
                            outsb[:, cft * NTOK + nch * 512: cft * NTOK + (nch + 1) * 512],
                            psp[:], ACT.Identity, bias=bp_sb[:, cft:cft + 1], scale=1.0)
                for cft in range(2):
                    nc.sync.dma_start(out_d[wi, cft], outsb[:, cft * NTOK:(cft + 1) * NTOK])
            ph2.close()

    _dedup_ldweights(nc, mybir)
    _split_sync_waits(nc, mybir, maxw=1)
    return nc


def _host_prepost(x, w_qkv, b_qkv):
    """Window partition, routing."""
    xw = x.reshape(T, B, WT, GT, WH, GH, WW, GW, C) \
          .transpose(0, 1, 2, 4, 6, 3, 5, 7, 8).reshape(T, B, W, S, C)
    xbar = xw.mean(axis=(0, 3))                      # [B, W, C]
    q_reg = xbar @ w_qkv[:, :C] + b_qkv[:C]
    k_reg = xbar @ w_qkv[:, C:2 * C] + b_qkv[C:2 * C]
    a_r = np.einsum('bwc,bvc->bwv', q_reg, k_reg)
    routing_idx = np.argsort(-a_r, axis=-1)[:, :, :TOPK]   # [B, W, TOPK]
    return xw, routing_idx


def _hi_lo16(a):
    hi = a.astype(ml_dtypes.float16 if hasattr(ml_dtypes, 'float16') else np.float16)
    hi = a.astype(np.float16)
    lo = (a - hi.astype(np.float32)).astype(np.float16)
    return hi, lo


def _hi_lo_bf(a):
    hi = a.astype(ml_dtypes.bfloat16)
    lo = (a - hi.astype(np.float32)).astype(ml_dtypes.bfloat16)
    return hi, lo


def kernel(x, w_qkv, b_qkv, w_proj, b_proj):
    x = np.ascontiguousarray(np.asarray(x, dtype=np.float32))
    w_qkv = np.asarray(w_qkv, dtype=np.float32)
    b_qkv = np.asarray(b_qkv, dtype=np.float32)
    w_proj = np.asarray(w_proj, dtype=np.float32)
    b_proj = np.asarray(b_proj, dtype=np.float32)

    xw, routing_idx = _host_prepost(x, w_qkv, b_qkv)

    key = (routing_idx.tobytes(), QKV_MODE, PJ_DT_NAME)
    if key not in _prog_cache:
        _prog_cache.clear()
        _prog_cache[key] = _build_program(routing_idx)
    nc = _prog_cache[key]

    split3 = QKV_MODE == "f16x3"
    np_qkv = np.float16 if split3 else np.float32

    # weights (shared across cores)
    wkv_half = (0.5 * w_qkv[:, C:]).astype(np.float32)
    wq_half = (0.5 * w_qkv[:, :C]).astype(np.float32)
    if split3:
        wkv_hi, wkv_lo = _hi_lo16(wkv_half)
        wkv_arr = np.stack([wkv_hi, wkv_lo]).reshape(2, 2, 128, 512)
        wq_hi, wq_lo = _hi_lo16(wq_half)
        wq_arr = np.stack([wq_hi, wq_lo]).reshape(2, 2, 128, 256)
    else:
        wkv_arr = wkv_half.reshape(1, 2, 128, 512)
        wq_arr = wq_half.reshape(1, 2, 128, 256)

    bkv_hi, bkv_lo = _hi_lo_bf((0.5 * b_qkv[C:]).astype(np.float32))
    bkv_arr = np.stack([bkv_hi, bkv_lo]).reshape(2, 512)
    bq_hi, bq_lo = _hi_lo_bf((0.5 * b_qkv[:C]).astype(np.float32))
    bq_arr = np.stack([bq_hi, bq_lo]).reshape(2, 256)

    wp = (SCALE * w_proj).reshape(2, 128, 256).astype(np.float32)
    bp = b_proj.reshape(2, 128, 1).astype(np.float32)

    in_maps = []
    for r in range(NCORES):
        b_of, wg = r // 4, r % 4
        xwc = xw[:, b_of, wg * 8:(wg + 1) * 8]              # [T, 8, S, C]
        xl32 = np.ascontiguousarray(
            xwc.transpose(1, 3, 0, 2).reshape(NW, 2, 128, NTOK))
        if split3:
            xhi = xl32.astype(np.float16)
            xlo = (xl32 - xhi.astype(np.float32)).astype(np.float16)
            xl = np.stack([xhi, xlo], axis=1)               # [NW, 2, 2, 128, NTOK]
        else:
            xl = xl32.reshape(NW, 1, 2, 128, NTOK)
        in_maps.append({
            "x_in": xl,
            "wkv_in": wkv_arr.astype(np_qkv), "wq_in": wq_arr.astype(np_qkv),
            "bkv_b_in": bkv_arr, "bq_b_in": bq_arr,
            "wp_in": wp, "bp_in": bp,
        })

    from concourse.bass_utils import run_bass_kernel_spmd
    res = run_bass_kernel_spmd(nc, in_maps, core_ids=list(range(NCORES)))

    # assemble output
    yw = np.empty((T, B, W, S, C), dtype=np.float32)
    for r in range(NCORES):
        b_of, wg = r // 4, r % 4
        o = res.results[r]["out_d"]                          # [NW, 2, 128, NTOK]
        o = o.reshape(NW, 2, 128, T, S).transpose(0, 3, 4, 1, 2).reshape(NW, T, S, C)
        for wl in range(NW):
            yw[:, b_of, wg * 8 + wl] = o[wl]

    y = yw.reshape(T, B, WT, WH, WW, GT, GH, GW, C) \
          .transpose(0, 1, 2, 5, 3, 6, 4, 7, 8).reshape(T, B, Lt, Lh, Lw, C)
    return y



# revision 51
# speedup vs baseline: 1.6097x; 1.6097x over previous
"""BiLevelRoutingAttention Trainium2 kernel (8-core SPMD), v6.

Sharding: core r handles batch b = r//4 and windows w in [ (r%4)*8, (r%4)*8+8 ).
Routing (region top-k) is computed on host via linearity of the mean.

Design notes (evolution from the 3-pass f16 baseline at 242us):
 - Single-pass fp16 QKV matmuls (~1e-3 rel err, budget 2e-2).
 - x loaded into SBUF once (resident); no phase-2 reload.
 - Phase 1 iterates t OUTER / window-pair INNER: at each step the engines see
   4 independent LIF chains (window pairs), so no engine serializes on the
   t-recurrence. R(wi, t) is issued per step (it only needs step-t spikes).
 - Window-PAIR batching: all LIF elementwise ops process two windows per
   instruction ([128, 2048] / [128,1024] ops) to amortize the ~150-300ns
   per-op dispatch + semaphore overhead that otherwise dominates.
 - Engine balance: PE does qkv/R/attn/proj + kv-bias matmuls; DVE runs spikes
   (4x tensor_scalar), lt/mult state, sh0/w0 psum adds; gpsimd (Pool) takes
   the sh1/w1 psum adds; ACT does t0 copies, kv spikes (saturated sigmoid:
   exactly 0/1 in f16 outside a ~2e-4 band), psr/attn/out psum drains (split
   with DVE in phase 2).
 - The R AllGather is split in two halves: t{0,1} fires mid-phase-1 (fully
   hidden), t{2,3} at the end, covered by the deferred last pair's q chain.
 - attn + proj in bf16 (attn counts <= 65536 exact in f32 psum, ~2e-3 after
   rounding), out DMA'd as bf16.

Spike arithmetic exact: spikes {0,1} f16; R counts <= 256 (f16-exact);
kv sums <= 2048 (f16-exact); attn accumulated in f32 PSUM.

Env knobs: KSPKKV = act|dve|pool, KSPKQ = dve|pool, KGAP = int.
"""

import numpy as np
import ml_dtypes
import os as _os

# ---- problem constants (hardcoded per contract) ----
T, B, Lt, Lh, Lw, C = 4, 2, 8, 32, 32, 256
WT, WH, WW = 2, 4, 4
W = WT * WH * WW            # 32 windows
GT, GH, GW = Lt // WT, Lh // WH, Lw // WW
S = GT * GH * GW            # 256 tokens per window
H, D = 8, C // 8            # 8 heads, 32 dim
TOPK = 8
SCALE = float(D) ** -0.5
NCORES = 8
NW = 8                      # windows per core
NTOK = T * S                # 1024 token-instances per window

SPKKV_ENG = _os.environ.get("KSPKKV", "act")
SPKQ_ENG = _os.environ.get("KSPKQ", "pool")
GATHER_POOL_EVERY = int(_os.environ.get("KGAP", "9999"))

_prog_cache = {}


def _dedup_ldweights(nc, mybir):
    """Drop an InstLdweights whose stationary operand is byte-identical to the
    immediately preceding PE Ldweights with only Matmults in between (the PE
    weight slot still holds the same data). Waits/updates are folded into the
    next PE instruction."""
    ndrop = 0
    for bb in nc.main_func.blocks:
        new_list = []
        last_sig = None
        pending = None   # (waits, updates) from a dropped ldw
        for ins in bb.instructions:
            tn = type(ins).__name__
            if ins.engine != mybir.EngineType.PE:
                new_list.append(ins)
                continue
            if pending is not None and tn in ("InstLdweights", "InstMatmult"):
                si = ins.sync_info
                w = list(si.on_wait) if si and si.on_wait else []
                u = list(si.on_update) if si and si.on_update else []
                ins.sync_info = mybir.SyncInfo(on_wait=pending[0] + w,
                                               on_update=pending[1] + u)
                pending = None
            if tn == "InstLdweights":
                try:
                    ap = ins.ins[0]
                    sig = repr(ap)
                except Exception:
                    sig = None
                if sig is not None and sig == last_sig:
                    si = ins.sync_info
                    w = list(si.on_wait) if si and si.on_wait else []
                    u = list(si.on_update) if si and si.on_update else []
                    pending = (w, u)
                    ndrop += 1
                    continue
                last_sig = sig
            elif tn != "InstMatmult":
                last_sig = None
            new_list.append(ins)
        assert pending is None or not (pending[0] or pending[1])
        bb.instructions[:] = new_list
    return ndrop


def _split_sync_waits(nc, mybir, maxw=1):
    """walrus in this container rejects >1 sync wait per instruction; split
    excess waits onto NoOp instructions inserted just before."""
    for bb in nc.main_func.blocks:
        new_list = []
        for ins in bb.instructions:
            si = ins.sync_info
            waits = list(si.on_wait) if si is not None and si.on_wait else []
            if len(waits) > maxw:
                extra = waits[:-maxw]
                keep = waits[-maxw:]
                idx = 0
                while extra:
                    chunk, extra = extra[:maxw], extra[maxw:]
                    nop = mybir.InstNoOp(name=f"{ins.name}-wsplit{idx}", ins=[], outs=[])
                    nop.engine = ins.engine
                    nop.sync_info = mybir.SyncInfo(on_wait=chunk, on_update=[])
                    new_list.append(nop)
                    idx += 1
                ins.sync_info = mybir.SyncInfo(
                    on_wait=keep,
                    on_update=list(si.on_update) if si.on_update else [],
                )
            new_list.append(ins)
        bb.instructions[:] = new_list
    return nc


def _build_program(routing_idx, sim_mode=False):
    """routing_idx: [B, W, TOPK] int array (host-computed). Returns nc.
    sim_mode: no collective / no If-chain (single-core TimelineSim)."""
    import contextlib
    import concourse.bass as bass
    import concourse.mybir as mybir
    import concourse.tile as tile

    f32 = mybir.dt.float32
    f16 = mybir.dt.float16
    bf16 = mybir.dt.bfloat16
    ALU = mybir.AluOpType
    ACT = mybir.ActivationFunctionType

    nc = bass.Bass(num_devices=NCORES)
    spkkv_eng = getattr(nc, {"pool": "gpsimd", "dve": "vector", "act": "vector"}[SPKKV_ENG])
    spkq_eng = getattr(nc, {"pool": "gpsimd", "dve": "vector"}[SPKQ_ENG])

    NP = NW // 2  # window pairs

    # ---- I/O ----
    x_in = nc.dram_tensor("x_in", [NW, 2, 128, NTOK], f16, kind="ExternalInput")
    wkv_in = nc.dram_tensor("wkv_in", [2, 128, 512], f16, kind="ExternalInput")
    wq_in = nc.dram_tensor("wq_in", [2, 128, 256], f16, kind="ExternalInput")
    bkv_in = nc.dram_tensor("bkv_in", [1, 512], f16, kind="ExternalInput")
    bq_in = nc.dram_tensor("bq_in", [128, 2], f32, kind="ExternalInput")
    wp_in = nc.dram_tensor("wp_in", [2, 128, 256], bf16, kind="ExternalInput")
    bp_in = nc.dram_tensor("bp_in", [2, 128, 1], f32, kind="ExternalInput")
    ident_in = nc.dram_tensor("ident_in", [128, 128], f16, kind="ExternalInput")
    out_d = nc.dram_tensor("out_d", [NW, 2, 128, NTOK], bf16, kind="ExternalOutput")

    with tile.TileContext(nc) as tc:
        with (
            tc.tile_pool(name="const", bufs=1) as constp,
            tc.tile_pool(name="hbuf", bufs=4) as h_p,
            tc.tile_pool(name="skv", bufs=6) as skv_p,
            tc.tile_pool(name="state", bufs=2) as st_p,
            tc.tile_pool(name="persist", bufs=1) as pers_p,
            tc.tile_pool(name="attn", bufs=2) as attn_p,
            tc.tile_pool(name="outs", bufs=2) as out_p,
            tc.tile_pool(name="dram", bufs=1, space="DRAM") as dram_p,
        ):
            # ---- constants / weights ----
            wkv_sb = constp.tile([128, 2 * 512], f16)
            wq_sb = constp.tile([128, 2 * 256], f16)
            bkv_sb = constp.tile([1, 512], f16)
            x_all = constp.tile([128, NW * 2 * NTOK], f16)  # cols (wi, kc, tok)
            # order startup DMAs so window-0 compute can begin ASAP
            for kc in range(2):
                nc.sync.dma_start(wkv_sb[:, kc * 512:(kc + 1) * 512], wkv_in[kc])
            nc.sync.dma_start(bkv_sb[:], bkv_in[:])
            for wi in range(2):
                for kc in range(2):
                    nc.sync.dma_start(
                        x_all[:, (wi * 2 + kc) * NTOK:(wi * 2 + kc + 1) * NTOK],
                        x_in[wi, kc])
            wp_sb = constp.tile([128, 2 * 256], bf16)
            for kc in range(2):
                nc.sync.dma_start(wq_sb[:, kc * 256:(kc + 1) * 256], wq_in[kc])
                nc.sync.dma_start(wp_sb[:, kc * 256:(kc + 1) * 256], wp_in[kc])
            bq_sb = constp.tile([128, 2], f32)
            nc.sync.dma_start(bq_sb[:], bq_in[:])
            bp_sb = constp.tile([128, 2], f32)
            for ftc in range(2):
                nc.sync.dma_start(bp_sb[:, ftc:ftc + 1], bp_in[ftc])
            ones1 = constp.tile([1, 128], f16)
            nc.vector.memset(ones1[:], 1.0)
            negk = constp.tile([128, 1], f32)
            nc.vector.memset(negk[:], -4096.0)
            ident = constp.tile([128, 128], f16)
            nc.sync.dma_start(ident[:], ident_in[:])
            for wi in range(2, NW):
                for kc in range(2):
                    nc.sync.dma_start(
                        x_all[:, (wi * 2 + kc) * NTOK:(wi * 2 + kc + 1) * NTOK],
                        x_in[wi, kc])

            def xsl(wi, kc, lo, hi):
                return x_all[:, (wi * 2 + kc) * NTOK + lo:(wi * 2 + kc) * NTOK + hi]

            # persistent across phases
            # r_loc cols: (t4, slab2, wi8, e32) - t-major so the collective
            # can ship t{0,1,2} early and only t{3} after the last step
            r_loc = pers_p.tile([128, 2048], f16)
            # r_all cols: part1 (rk4, t3, slab2, w8, e32) then part2 (rk4, slab2, w8, e32)
            r_all = pers_p.tile([128, 8192], f16)
            kv_w = [pers_p.tile([128, 256], f16, name=f"kvw{i}") for i in range(NW)]
            sq_all = pers_p.tile([128, NW * 2048], f16)    # q spikes (wi, f, t, e)
            vkv_w = [pers_p.tile([128, 2048], f16, name=f"vkv{i}") for i in range(NP)]
            vq_w = [pers_p.tile([128, 1024], f16, name=f"vq{i}") for i in range(NP)]

            # ================= phase 1: kv + q qkv/LIF/R, t-outer, pair ops ====
            ph1 = contextlib.ExitStack()
            ps_a = ph1.enter_context(tc.tile_pool(name="psA", bufs=2, space="PSUM"))
            ps_r = ph1.enter_context(tc.tile_pool(name="psr", bufs=2, space="PSUM"))
            ps_b = ph1.enter_context(tc.tile_pool(name="psB", bufs=1, space="PSUM"))
            QDEFER_PAIRS = int(_os.environ.get("KQD", "1"))
            sq_v = sq_all[:].rearrange("p (w f t e) -> p w f t e", w=NW, f=2, t=4, e=256)

            def emit_kv(nc, pi, t):
                """kv matmuls + LIF for window pair (2pi, 2pi+1) at step t.
                Returns the pair spike tile [128, (sh2, w2, feat512)].

                gpsimd cannot touch PSUM, so: sh0 carry-add on DVE; sh1 carry
                enters PSUM via a PE identity-matmul accumulate and the sum is
                drained to f16 by ACT. gpsimd gets SBUF-only state work."""
                vkv = vkv_w[pi]
                pss = []
                for sh in range(2):
                    st = t * 2 + sh
                    ps = ps_a.tile([128, 1024], f32, tag="psa", name="ps")
                    pss.append(ps)
                    for w in range(2):
                        for kc in range(2):
                            nc.tensor.matmul(
                                ps[:, w * 512:(w + 1) * 512],
                                lhsT=xsl(2 * pi + w, kc, st * 128, (st + 1) * 128),
                                rhs=wkv_sb[:, kc * 512:(kc + 1) * 512],
                                start=(kc == 0), stop=False)
                    # bias matmuls adjacent (ldweights of ones1 dedups)
                    bias_last = (sh == 0) or (t == 0)
                    for w in range(2):
                        nc.tensor.matmul(ps[:, w * 512:(w + 1) * 512],
                                         lhsT=ones1[:], rhs=bkv_sb[:],
                                         start=False, stop=bias_last)
                    if not bias_last:
                        # sh1 carry-add on PE: ps += I @ vkv_sh1
                        for w in range(2):
                            nc.tensor.matmul(
                                ps[:, w * 512:(w + 1) * 512],
                                lhsT=ident[:],
                                rhs=vkv[:, 1024 + w * 512:1024 + (w + 1) * 512],
                                start=False, stop=(w == 1))
                skt = skv_p.tile([128, 2048], f16, tag="skt")
                hsb = h_p.tile([128, 2048], f16, tag="hkv")
                # sh0: DVE add (or ACT copy at t=0); sh1: ACT drain (carry
                # already accumulated in PSUM by the PE)
                if t == 0:
                    nc.scalar.activation(hsb[:, 0:1024], pss[0][:], ACT.Copy,
                                         bias=0.0, scale=1.0)
                else:
                    nc.vector.tensor_tensor(hsb[:, 0:1024], pss[0][:],
                                            vkv[:, 0:1024], op=ALU.add)
                nc.scalar.activation(hsb[:, 1024:2048], pss[1][:], ACT.Copy,
                                     bias=0.0, scale=1.0)
                # spike: saturated sigmoid(4096*(h-1)) on ACT is exactly 0/1
                # in f16 outside a ~2e-4-wide band around threshold
                if SPKKV_ENG == "act":
                    nc.scalar.activation(skt[:], hsb[:], ACT.Sigmoid,
                                         bias=negk[:, 0:1], scale=4096.0)
                else:
                    spkkv_eng.tensor_scalar(skt[:], hsb[:], 1.0, None, ALU.is_ge)
                if t < T - 1:
                    # state: lt on DVE (both halves); mult sh0 on DVE, sh1 on
                    # gpsimd (all SBUF)
                    lt = st_p.tile([128, 2048], f16, tag="ltkv")
                    for sh in range(2):
                        sl = slice(sh * 1024, (sh + 1) * 1024)
                        nc.vector.tensor_scalar(lt[:, sl], hsb[:, sl], 1.0, 0.5,
                                                ALU.is_lt, ALU.mult)
                        eng = nc.vector if sh == 0 else nc.gpsimd
                        eng.tensor_tensor(vkv[:, sl], hsb[:, sl], lt[:, sl],
                                          op=ALU.mult)
                return skt

            def emit_r(nc, pi, t, skt):
                # one [128,128] psum tile per (pair, t): quadrant (w, slab),
                # drained by a single strided ACT copy into r_loc
                psr = ps_r.tile([128, 128], f32, tag="psr", name="psr")
                for w in range(2):
                    for slab in range(2):
                        reg = psr[:, (w * 2 + slab) * 32:(w * 2 + slab + 1) * 32]
                        for hl in range(4):
                            h = slab * 4 + hl
                            for sh in range(2):
                                base = sh * 1024 + w * 512
                                nc.tensor.matmul(
                                    reg[32 * hl:32 * (hl + 1), :],
                                    lhsT=skt[:, base + h * 32: base + (h + 1) * 32],
                                    rhs=skt[:, base + 256 + h * 32: base + 256 + (h + 1) * 32],
                                    start=(sh == 0), stop=(sh == 1),
                                    tile_position=(0, 32 * hl),
                                )
                psr_v = psr[:].rearrange("p (w s e) -> p w s e", w=2, s=2, e=32)
                dst = r_loc[:].rearrange("p (t s wi e) -> p t wi s e",
                                         t=4, s=2, wi=8, e=32)[:, t, 2 * pi:2 * pi + 2]
                nc.scalar.activation(dst, psr_v, ACT.Copy, bias=0.0, scale=1.0)

            def emit_q(nc, pi, t):
                vq = vq_w[pi]
                psq = ps_b.tile([128, 1024], f32, tag="psb", name="psq")
                for w in range(2):
                    for ftc in range(2):
                        for kc in range(2):
                            nc.tensor.matmul(
                                psq[:, w * 512 + ftc * 256: w * 512 + (ftc + 1) * 256],
                                lhsT=wq_sb[:, kc * 256 + ftc * 128:
                                           kc * 256 + (ftc + 1) * 128],
                                rhs=xsl(2 * pi + w, kc, t * 256, (t + 1) * 256),
                                start=(kc == 0), stop=(kc == 1))
                hq = h_p.tile([128, 1024], f16, tag="hq")
                hq_v = hq[:].rearrange("p (w f e) -> p w f e", w=2, f=2, e=256)
                vq_v = vq[:].rearrange("p (w f e) -> p w f e", w=2, f=2, e=256)
                if t == 0:
                    for w in range(2):
                        for f in range(2):
                            nc.scalar.activation(
                                hq_v[:, w, f, :],
                                psq[:, w * 512 + f * 256: w * 512 + (f + 1) * 256],
                                ACT.Identity, bias=bq_sb[:, f:f + 1], scale=1.0)
                else:
                    nc.vector.tensor_tensor(hq[:], psq[:], vq[:], op=ALU.add)
                spkq_eng.tensor_scalar(sq_v[:, 2 * pi:2 * pi + 2, :, t, :], hq_v,
                                       1.0, None, ALU.is_ge)
                if t < T - 1:
                    ltq = st_p.tile([128, 1024], f16, tag="ltq")
                    nc.vector.tensor_scalar(ltq[:], hq[:], 1.0, 0.5,
                                            ALU.is_lt, ALU.mult)
                    nc.vector.tensor_tensor(vq[:], hq[:], ltq[:], op=ALU.mult)
                    # carry must include +0.5*b_q (bias re-enters h each
                    # step; per-partition since q is feature-major)
                    for w in range(2):
                        for f in range(2):
                            nc.vector.tensor_scalar(
                                vq_v[:, w, f, :], vq_v[:, w, f, :],
                                bq_sb[:, f:f + 1], None, ALU.add)

            # collective buffers (two parts: t{0,1,2} and t{3})
            rb_in0 = dram_p.tile([128, 1536], f16)
            rb_out0 = dram_p.tile([4, 128, 1536], f16)
            rb_in1 = dram_p.tile([128, 512], f16)
            rb_out1 = dram_p.tile([4, 128, 512], f16)

            def emit_collective(nc, part):
                rbi = rb_in0 if part == 0 else rb_in1
                rbo = rb_out0 if part == 0 else rb_out1
                lo, sz, base = (0, 1536, 0) if part == 0 else (1536, 512, 6144)
                nc.sync.dma_start(rbi[:], r_loc[:, lo:lo + sz])
                if sim_mode:
                    for rk in range(4):
                        nc.sync.dma_start(rbo[rk], rbi[:])
                else:
                    nc.gpsimd.collective_compute(
                        "AllGather",
                        mybir.AluOpType.bypass,
                        replica_groups=[[0, 1, 2, 3], [4, 5, 6, 7]],
                        ins=[rbi[:].opt()],
                        outs=[rbo[:].opt()],
                    )
                # single merged redistribute DMA (SP SEQ dispatch is ~700ns per
                # DMA and sits on the critical tail)
                nc.sync.dma_start(
                    r_all[:, base: base + 4 * sz].rearrange("p (k s) -> p k s",
                                                            k=4, s=sz),
                    rbo[:].rearrange("k p s -> p k s"))

            for t in range(T):
                skts = []
                for pi in range(NP):
                    skts.append(emit_kv(nc, pi, t))
                    if pi < NP - QDEFER_PAIRS:
                        emit_q(nc, pi, t)
                for pi in range(NP):
                    emit_r(nc, pi, t, skts[pi])
                if t == 2:
                    emit_collective(nc, 0)   # t{0,1,2}: fully hidden under t=3
            emit_collective(nc, 1)           # t{3}: small, covered by deferred q
            for t in range(T):               # t-outer: parallel deferred chains
                for pi in range(NP - QDEFER_PAIRS, NP):
                    emit_q(nc, pi, t)
            ph1.close()

            # ============ phase 2: kv sums, attention, proj =====================
            ph2 = contextlib.ExitStack()
            # gather views: dims (rk, w) select j; kv_w cols are (t4, slab2, e32)
            # so part-1 (t<3) is kv_w[:, 0:192] and part-2 (t=3) kv_w[:, 192:256]
            rA = r_all[:, 0:6144].rearrange("p (k t a w e) -> p k w t a e",
                                            k=4, t=3, a=2, w=8, e=32)
            rB = r_all[:, 6144:8192].rearrange("p (k a w e) -> p k w a e",
                                               k=4, a=2, w=8, e=32)
            # kv sums: routed gather baked per core, guarded by If on core id
            pid = None if sim_mode else nc.partition_id()
            for r in range(NCORES):
                if sim_mode and r != 0:
                    continue
                b_of = r // 4
                wg = r % 4
                with (contextlib.nullcontext() if sim_mode else tc.If(pid == r)):
                    for wl in range(NW):
                        wglob = wg * 8 + wl
                        idxs = [int(j) for j in routing_idx[b_of, wglob]]
                        eng = nc.gpsimd if (wl % GATHER_POOL_EVERY == GATHER_POOL_EVERY - 1) else nc.vector
                        for dst, rv in ((kv_w[wl][:, 0:192], rA),
                                        (kv_w[wl][:, 192:256], rB)):
                            eng.tensor_copy(dst, rv[:, idxs[0] // 8, idxs[0] % 8])
                            for j in idxs[1:]:
                                eng.tensor_tensor(
                                    dst, dst, rv[:, j // 8, j % 8], op=ALU.add)

            ps_at = ph2.enter_context(tc.tile_pool(name="psat", bufs=2, space="PSUM"))
            ps_pj = ph2.enter_context(tc.tile_pool(name="pspj", bufs=2, space="PSUM"))
            for wi in range(NW):
                attn = attn_p.tile([128, 2 * NTOK], bf16, tag="attn")
                for slab in range(2):
                    psa = ps_at.tile([128, 1024], f32, tag="psat", name="psa")
                    for t in range(T):
                        for hl in range(4):
                            nc.tensor.matmul(
                                psa[32 * hl:32 * (hl + 1), t * 256:(t + 1) * 256],
                                lhsT=kv_w[wi][32 * hl:32 * (hl + 1),
                                              t * 64 + slab * 32:
                                              t * 64 + (slab + 1) * 32],
                                rhs=sq_all[32 * hl:32 * (hl + 1),
                                           wi * 2048 + slab * NTOK + t * 256:
                                           wi * 2048 + slab * NTOK + (t + 1) * 256],
                                start=True, stop=True,
                                tile_position=(32 * hl, 32 * hl),
                            )
                    dst_at = attn[:, slab * NTOK:(slab + 1) * NTOK]
                    # alternate [128,1024] drains between ACT and idle DVE
                    if slab == 0:
                        nc.scalar.activation(dst_at, psa[:], ACT.Copy,
                                             bias=0.0, scale=1.0)
                    else:
                        nc.vector.tensor_copy(dst_at, psa[:])

                outsb = out_p.tile([128, 2 * NTOK], bf16, tag="outsb")
                for cft in range(2):
                    for nch in range(2):
                        psp = ps_pj.tile([128, 512], f32, tag="pspj", name="psp")
                        for kc in range(2):
                            nc.tensor.matmul(
                                psp[:],
                                lhsT=wp_sb[:, kc * 256 + cft * 128: kc * 256 + (cft + 1) * 128],
                                rhs=attn[:, kc * NTOK + nch * 512: kc * NTOK + (nch + 1) * 512],
                                start=(kc == 0), stop=(kc == 1),
                            )
                        dst_o = outsb[:, cft * NTOK + nch * 512:
                                      cft * NTOK + (nch + 1) * 512]
                        if nch == 0:
                            nc.scalar.activation(dst_o, psp[:], ACT.Identity,
                                                 bias=bp_sb[:, cft:cft + 1], scale=1.0)
                        else:
                            nc.vector.tensor_scalar(dst_o, psp[:],
                                                    bp_sb[:, cft:cft + 1], None,
                                                    ALU.add)
                for cft in range(2):
                    nc.sync.dma_start(out_d[wi, cft], outsb[:, cft * NTOK:(cft + 1) * NTOK])
            ph2.close()

    _dedup_ldweights(nc, mybir)
    _split_sync_waits(nc, mybir, maxw=1)
    return nc


def _host_prepost(x, w_qkv, b_qkv):
    """Window partition, routing."""
    xw = x.reshape(T, B, WT, GT, WH, GH, WW, GW, C) \
          .transpose(0, 1, 2, 4, 6, 3, 5, 7, 8).reshape(T, B, W, S, C)
    xbar = xw.mean(axis=(0, 3))                      # [B, W, C]
    q_reg = xbar @ w_qkv[:, :C] + b_qkv[:C]
    k_reg = xbar @ w_qkv[:, C:2 * C] + b_qkv[C:2 * C]
    a_r = np.einsum('bwc,bvc->bwv', q_reg, k_reg)
    routing_idx = np.argsort(-a_r, axis=-1)[:, :, :TOPK]   # [B, W, TOPK]
    return xw, routing_idx


def kernel(x, w_qkv, b_qkv, w_proj, b_proj):
    x = np.ascontiguousarray(np.asarray(x, dtype=np.float32))
    w_qkv = np.asarray(w_qkv, dtype=np.float32)
    b_qkv = np.asarray(b_qkv, dtype=np.float32)
    w_proj = np.asarray(w_proj, dtype=np.float32)
    b_proj = np.asarray(b_proj, dtype=np.float32)

    xw, routing_idx = _host_prepost(x, w_qkv, b_qkv)

    key = (routing_idx.tobytes(), SPKKV_ENG, SPKQ_ENG, GATHER_POOL_EVERY)
    if key not in _prog_cache:
        _prog_cache.clear()
        _prog_cache[key] = _build_program(routing_idx)
    nc = _prog_cache[key]

    # weights (shared across cores), all pre-halved for the LIF charge h=(v+x)/2
    wkv_arr = (0.5 * w_qkv[:, C:]).reshape(2, 128, 512).astype(np.float16)
    wq_arr = (0.5 * w_qkv[:, :C]).reshape(2, 128, 256).astype(np.float16)
    bkv_arr = (0.5 * b_qkv[C:]).reshape(1, 512).astype(np.float16)
    bq_arr = (0.5 * b_qkv[:C]).reshape(2, 128).T.astype(np.float32).copy()
    wp = (SCALE * w_proj).reshape(2, 128, 256).astype(ml_dtypes.bfloat16)
    bp = b_proj.reshape(2, 128, 1).astype(np.float32)
    ident = np.eye(128, dtype=np.float16)

    in_maps = []
    for r in range(NCORES):
        b_of, wg = r // 4, r % 4
        xwc = xw[:, b_of, wg * 8:(wg + 1) * 8]              # [T, 8, S, C]
        xl = np.ascontiguousarray(
            xwc.transpose(1, 3, 0, 2).reshape(NW, 2, 128, NTOK)).astype(np.float16)
        in_maps.append({
            "x_in": xl,
            "wkv_in": wkv_arr, "wq_in": wq_arr,
            "bkv_in": bkv_arr, "bq_in": bq_arr,
            "wp_in": wp, "bp_in": bp, "ident_in": ident,
        })

    from concourse.bass_utils import run_bass_kernel_spmd
    res = run_bass_kernel_spmd(nc, in_maps, core_ids=list(range(NCORES)))

    # assemble output
    yw = np.empty((T, B, W, S, C), dtype=np.float32)
    for r in range(NCORES):
        b_of, wg = r // 4, r % 4
        o = np.asarray(res.results[r]["out_d"]).astype(np.float32)  # [NW,2,128,NTOK]
        o = o.reshape(NW, 2, 128, T, S).transpose(0, 3, 4, 1, 2).reshape(NW, T, S, C)
        for wl in range(NW):
            yw[:, b_of, wg * 8 + wl] = o[wl]

    y = yw.reshape(T, B, WT, WH, WW, GT, GH, GW, C) \
          .transpose(0, 1, 2, 5, 3, 6, 4, 7, 8).reshape(T, B, Lt, Lh, Lw, C)
    return y


# revision 78
# speedup vs baseline: 1.8079x; 1.1232x over previous
"""BiLevelRoutingAttention Trainium2 kernel (8-core SPMD).

Sharding: core r handles batch b = r//4 and windows w in [ (r%4)*8, (r%4)*8+8 ).
Routing (region top-k) is computed on host via linearity of the mean.

Design notes (evolution from the 3-pass f16 baseline at 242.7us -> 134.2us):
 - Single-pass fp16 QKV matmuls (~1e-3 rel err, budget 2e-2).
 - x loaded into SBUF once (resident); no phase-2 reload.
 - Phase 1 iterates t OUTER / window-pair INNER: at each step the engines see
   4 independent LIF chains (window pairs), so no engine serializes on the
   t-recurrence. R(wi, t) is issued per step (it only needs step-t spikes).
 - Window-PAIR batching: all LIF elementwise ops process two windows per
   instruction ([128, 2048] / [128,1024] ops) to amortize the ~150-300ns
   per-op dispatch + semaphore overhead that otherwise dominates.
 - gpsimd cannot touch PSUM (BIR verifier), so: kv sh0 carry-add on DVE, kv
   sh1 carry injected into PSUM by a PE identity-matmul accumulate + ACT
   drain; gpsimd gets SBUF-only work (q spikes, kv sh1 state mult).
 - kv spikes via saturated ACT sigmoid(4096*(h-1)) (exactly 0/1 in f16
   outside a ~2e-4 band; same act-table set as Copy => no table reloads);
   t=0 spikes on DVE is_ge while ACT is busy with t0 drains.
 - The R AllGather is split t{0,1,2} / t{3}: the big part fires mid-phase-1
   (fully hidden), the t3 part at the end, covered by the deferred last
   pair's q chain. Redistribute is one merged DMA (SP dispatch ~700ns each).
 - Attention uses block-diagonal kv stationaries built on PE (zero-matmul +
   4 identity quadrant copies into PSUM, drained to SBUF f16): one matmul
   per (window, slab, t) contracts all 128 partitions, 4x fewer PE cycles
   than per-head 32-contract matmuls.
 - attn + proj in bf16 (attn counts <= 65536 exact in f32 psum, ~2e-3 after
   rounding), out DMA'd as bf16.

Spike arithmetic exact: spikes {0,1} f16; R counts <= 256 (f16-exact);
kv sums <= 2048 (f16-exact); attn accumulated in f32 PSUM.

Env knobs (defaults = tuned best): KSPKKV=act, KSPKQ=pool, KGAP=4, KQD=1,
KSPT=1, KT0=act, KHB/KSB/KST pool sizes.
"""

import numpy as np
import ml_dtypes
import os as _os

# ---- problem constants (hardcoded per contract) ----
T, B, Lt, Lh, Lw, C = 4, 2, 8, 32, 32, 256
WT, WH, WW = 2, 4, 4
W = WT * WH * WW            # 32 windows
GT, GH, GW = Lt // WT, Lh // WH, Lw // WW
S = GT * GH * GW            # 256 tokens per window
H, D = 8, C // 8            # 8 heads, 32 dim
TOPK = 8
SCALE = float(D) ** -0.5
NCORES = 8
NW = 8                      # windows per core
NTOK = T * S                # 1024 token-instances per window

SPKKV_ENG = _os.environ.get("KSPKKV", "act")
SPKQ_ENG = _os.environ.get("KSPKQ", "pool")
GATHER_POOL_EVERY = int(_os.environ.get("KGAP", "4"))

_prog_cache = {}


def _dedup_ldweights(nc, mybir):
    """Drop an InstLdweights whose stationary operand is byte-identical to the
    immediately preceding PE Ldweights with only Matmults in between (the PE
    weight slot still holds the same data). Waits/updates are folded into the
    next PE instruction."""
    ndrop = 0
    for bb in nc.main_func.blocks:
        new_list = []
        last_sig = None
        pending = None   # (waits, updates) from a dropped ldw
        for ins in bb.instructions:
            tn = type(ins).__name__
            if ins.engine != mybir.EngineType.PE:
                new_list.append(ins)
                continue
            if pending is not None and tn in ("InstLdweights", "InstMatmult"):
                si = ins.sync_info
                w = list(si.on_wait) if si and si.on_wait else []
                u = list(si.on_update) if si and si.on_update else []
                ins.sync_info = mybir.SyncInfo(on_wait=pending[0] + w,
                                               on_update=pending[1] + u)
                pending = None
            if tn == "InstLdweights":
                try:
                    ap = ins.ins[0]
                    sig = repr(ap)
                except Exception:
                    sig = None
                if sig is not None and sig == last_sig:
                    si = ins.sync_info
                    w = list(si.on_wait) if si and si.on_wait else []
                    u = list(si.on_update) if si and si.on_update else []
                    pending = (w, u)
                    ndrop += 1
                    continue
                last_sig = sig
            elif tn != "InstMatmult":
                last_sig = None
            new_list.append(ins)
        assert pending is None or not (pending[0] or pending[1])
        bb.instructions[:] = new_list
    return ndrop


def _split_sync_waits(nc, mybir, maxw=1):
    """walrus in this container rejects >1 sync wait per instruction; split
    excess waits onto NoOp instructions inserted just before."""
    for bb in nc.main_func.blocks:
        new_list = []
        for ins in bb.instructions:
            si = ins.sync_info
            waits = list(si.on_wait) if si is not None and si.on_wait else []
            if len(waits) > maxw:
                extra = waits[:-maxw]
                keep = waits[-maxw:]
                idx = 0
                while extra:
                    chunk, extra = extra[:maxw], extra[maxw:]
                    nop = mybir.InstNoOp(name=f"{ins.name}-wsplit{idx}", ins=[], outs=[])
                    nop.engine = ins.engine
                    nop.sync_info = mybir.SyncInfo(on_wait=chunk, on_update=[])
                    new_list.append(nop)
                    idx += 1
                ins.sync_info = mybir.SyncInfo(
                    on_wait=keep,
                    on_update=list(si.on_update) if si.on_update else [],
                )
            new_list.append(ins)
        bb.instructions[:] = new_list
    return nc


def _build_program(routing_idx, sim_mode=False):
    """routing_idx: [B, W, TOPK] int array (host-computed). Returns nc.
    sim_mode: no collective / no If-chain (single-core TimelineSim)."""
    import contextlib
    import concourse.bass as bass
    import concourse.mybir as mybir
    import concourse.tile as tile

    f32 = mybir.dt.float32
    f16 = mybir.dt.float16
    bf16 = mybir.dt.bfloat16
    ALU = mybir.AluOpType
    ACT = mybir.ActivationFunctionType

    nc = bass.Bass(num_devices=NCORES)
    spkkv_eng = getattr(nc, {"pool": "gpsimd", "dve": "vector", "act": "vector"}[SPKKV_ENG])
    spkq_eng = getattr(nc, {"pool": "gpsimd", "dve": "vector"}[SPKQ_ENG])

    NP = NW // 2  # window pairs

    # ---- I/O ----
    x_in = nc.dram_tensor("x_in", [NW, 2, 128, NTOK], f16, kind="ExternalInput")
    wkv_in = nc.dram_tensor("wkv_in", [2, 128, 512], f16, kind="ExternalInput")
    wq_in = nc.dram_tensor("wq_in", [2, 128, 256], f16, kind="ExternalInput")
    bkv_in = nc.dram_tensor("bkv_in", [1, 512], f16, kind="ExternalInput")
    bq_in = nc.dram_tensor("bq_in", [128, 2], f32, kind="ExternalInput")
    wp_in = nc.dram_tensor("wp_in", [2, 128, 256], bf16, kind="ExternalInput")
    bp_in = nc.dram_tensor("bp_in", [2, 128, 1], f32, kind="ExternalInput")
    ident_in = nc.dram_tensor("ident_in", [128, 128], f16, kind="ExternalInput")
    out_d = nc.dram_tensor("out_d", [NW, 2, 128, NTOK], bf16, kind="ExternalOutput")

    with tile.TileContext(nc) as tc:
        with (
            tc.tile_pool(name="const", bufs=1) as constp,
            tc.tile_pool(name="hbuf", bufs=4) as h_p,
            tc.tile_pool(name="skv", bufs=6) as skv_p,
            tc.tile_pool(name="state", bufs=2) as st_p,
            tc.tile_pool(name="persist", bufs=1) as pers_p,
            tc.tile_pool(name="attn", bufs=2) as attn_p,
            tc.tile_pool(name="outs", bufs=2) as out_p,
            tc.tile_pool(name="dram", bufs=1, space="DRAM") as dram_p,
        ):
            # ---- constants / weights ----
            wkv_sb = constp.tile([128, 2 * 512], f16)
            wq_sb = constp.tile([128, 2 * 256], f16)
            bkv_sb = constp.tile([1, 512], f16)
            x_all = constp.tile([128, NW * 2 * NTOK], f16)  # cols (wi, kc, tok)
            # order startup DMAs so window-0 compute can begin ASAP
            for kc in range(2):
                nc.sync.dma_start(wkv_sb[:, kc * 512:(kc + 1) * 512], wkv_in[kc])
            nc.sync.dma_start(bkv_sb[:], bkv_in[:])
            for wi in range(2):
                for kc in range(2):
                    nc.sync.dma_start(
                        x_all[:, (wi * 2 + kc) * NTOK:(wi * 2 + kc + 1) * NTOK],
                        x_in[wi, kc])
            wp_sb = constp.tile([128, 2 * 256], bf16)
            for kc in range(2):
                nc.sync.dma_start(wq_sb[:, kc * 256:(kc + 1) * 256], wq_in[kc])
                nc.sync.dma_start(wp_sb[:, kc * 256:(kc + 1) * 256], wp_in[kc])
            bq_sb = constp.tile([128, 2], f32)
            nc.sync.dma_start(bq_sb[:], bq_in[:])
            bp_sb = constp.tile([128, 2], f32)
            for ftc in range(2):
                nc.sync.dma_start(bp_sb[:, ftc:ftc + 1], bp_in[ftc])
            ones1 = constp.tile([1, 128], f16)
            nc.vector.memset(ones1[:], 1.0)
            negk = constp.tile([128, 1], f32)
            nc.vector.memset(negk[:], -4096.0)
            ident = constp.tile([128, 128], f16)
            nc.sync.dma_start(ident[:], ident_in[:])
            for wi in range(2, NW):
                for kc in range(2):
                    nc.sync.dma_start(
                        x_all[:, (wi * 2 + kc) * NTOK:(wi * 2 + kc + 1) * NTOK],
                        x_in[wi, kc])

            def xsl(wi, kc, lo, hi):
                return x_all[:, (wi * 2 + kc) * NTOK + lo:(wi * 2 + kc) * NTOK + hi]

            # persistent across phases
            # r_loc cols: (t4, slab2, wi8, e32) - t-major so the collective
            # can ship t{0,1,2} early and only t{3} after the last step
            r_loc = pers_p.tile([128, 2048], f16)
            # r_all cols: part1 (rk4, t3, slab2, w8, e32) then part2 (rk4, slab2, w8, e32)
            r_all = pers_p.tile([128, 8192], f16)
            kv_w = [pers_p.tile([128, 256], f16, name=f"kvw{i}") for i in range(NW)]
            sq_all = pers_p.tile([128, NW * 2048], f16)    # q spikes (wi, f, t, e)
            vkv_w = [pers_p.tile([128, 2048], f16, name=f"vkv{i}") for i in range(NP)]
            vq_w = [pers_p.tile([128, 1024], f16, name=f"vq{i}") for i in range(NP)]

            # ================= phase 1: kv + q qkv/LIF/R, t-outer, pair ops ====
            ph1 = contextlib.ExitStack()
            ps_a = ph1.enter_context(tc.tile_pool(name="psA", bufs=2, space="PSUM"))
            ps_r = ph1.enter_context(tc.tile_pool(name="psr", bufs=2, space="PSUM"))
            ps_b = ph1.enter_context(tc.tile_pool(name="psB", bufs=1, space="PSUM"))
            QDEFER_PAIRS = int(_os.environ.get("KQD", "1"))
            sq_v = sq_all[:].rearrange("p (w f t e) -> p w f t e", w=NW, f=2, t=4, e=256)

            def emit_kv(nc, pi, t):
                """kv matmuls + LIF for window pair (2pi, 2pi+1) at step t.
                Returns the pair spike tile [128, (sh2, w2, feat512)].

                gpsimd cannot touch PSUM, so: sh0 carry-add on DVE; sh1 carry
                enters PSUM via a PE identity-matmul accumulate and the sum is
                drained to f16 by ACT. gpsimd gets SBUF-only state work."""
                vkv = vkv_w[pi]
                pss = []
                for sh in range(2):
                    st = t * 2 + sh
                    ps = ps_a.tile([128, 1024], f32, tag="psa", name="ps")
                    pss.append(ps)
                    for w in range(2):
                        for kc in range(2):
                            nc.tensor.matmul(
                                ps[:, w * 512:(w + 1) * 512],
                                lhsT=xsl(2 * pi + w, kc, st * 128, (st + 1) * 128),
                                rhs=wkv_sb[:, kc * 512:(kc + 1) * 512],
                                start=(kc == 0), stop=False)
                    # bias matmuls adjacent (ldweights of ones1 dedups)
                    bias_last = (sh == 0) or (t == 0)
                    for w in range(2):
                        nc.tensor.matmul(ps[:, w * 512:(w + 1) * 512],
                                         lhsT=ones1[:], rhs=bkv_sb[:],
                                         start=False, stop=bias_last)
                    if not bias_last:
                        # sh1 carry-add on PE: ps += I @ vkv_sh1
                        for w in range(2):
                            nc.tensor.matmul(
                                ps[:, w * 512:(w + 1) * 512],
                                lhsT=ident[:],
                                rhs=vkv[:, 1024 + w * 512:1024 + (w + 1) * 512],
                                start=False, stop=(w == 1))
                skt = skv_p.tile([128, 2048], f16, tag="skt")
                hsb = h_p.tile([128, 2048], f16, tag="hkv")
                # sh0: DVE add (or DVE copy at t=0 - ACT is saturated early);
                # sh1: ACT drain (carry already accumulated in PSUM by the PE)
                if t == 0:
                    nc.vector.tensor_copy(hsb[:, 0:1024], pss[0][:])
                else:
                    nc.vector.tensor_tensor(hsb[:, 0:1024], pss[0][:],
                                            vkv[:, 0:1024], op=ALU.add)
                nc.scalar.activation(hsb[:, 1024:2048], pss[1][:], ACT.Copy,
                                     bias=0.0, scale=1.0)
                # spike: saturated sigmoid(4096*(h-1)) on ACT is exactly 0/1
                # in f16 outside a ~2e-4-wide band around threshold. Early
                # steps (t<2) run is_ge on DVE instead: ACT is the early
                # bottleneck while DVE still has slack.
                if SPKKV_ENG == "act" and t >= 2:
                    nc.scalar.activation(skt[:], hsb[:], ACT.Sigmoid,
                                         bias=negk[:, 0:1], scale=4096.0)
                else:
                    nc.vector.tensor_scalar(skt[:], hsb[:], 1.0, None, ALU.is_ge)
                if t < T - 1:
                    # state: lt on DVE (both halves); mult sh0 on DVE, sh1 on
                    # gpsimd (all SBUF)
                    lt = st_p.tile([128, 2048], f16, tag="ltkv")
                    for sh in range(2):
                        sl = slice(sh * 1024, (sh + 1) * 1024)
                        nc.vector.tensor_scalar(lt[:, sl], hsb[:, sl], 1.0, 0.5,
                                                ALU.is_lt, ALU.mult)
                        eng = nc.vector if sh == 0 else nc.gpsimd
                        eng.tensor_tensor(vkv[:, sl], hsb[:, sl], lt[:, sl],
                                          op=ALU.mult)
                return skt

            def emit_r(nc, pi, t, skt):
                # one [128,128] psum tile per (pair, t): quadrant (w, slab),
                # drained by a single strided ACT copy into r_loc
                psr = ps_r.tile([128, 128], f32, tag="psr", name="psr")
                for w in range(2):
                    for slab in range(2):
                        reg = psr[:, (w * 2 + slab) * 32:(w * 2 + slab + 1) * 32]
                        for hl in range(4):
                            h = slab * 4 + hl
                            for sh in range(2):
                                base = sh * 1024 + w * 512
                                nc.tensor.matmul(
                                    reg[32 * hl:32 * (hl + 1), :],
                                    lhsT=skt[:, base + h * 32: base + (h + 1) * 32],
                                    rhs=skt[:, base + 256 + h * 32: base + 256 + (h + 1) * 32],
                                    start=(sh == 0), stop=(sh == 1),
                                    tile_position=(0, 32 * hl),
                                )
                psr_v = psr[:].rearrange("p (w s e) -> p w s e", w=2, s=2, e=32)
                dst = r_loc[:].rearrange("p (t s wi e) -> p t wi s e",
                                         t=4, s=2, wi=8, e=32)[:, t, 2 * pi:2 * pi + 2]
                nc.scalar.activation(dst, psr_v, ACT.Copy, bias=0.0, scale=1.0)

            def emit_q(nc, pi, t):
                vq = vq_w[pi]
                psq = ps_b.tile([128, 1024], f32, tag="psb", name="psq")
                for w in range(2):
                    for ftc in range(2):
                        for kc in range(2):
                            nc.tensor.matmul(
                                psq[:, w * 512 + ftc * 256: w * 512 + (ftc + 1) * 256],
                                lhsT=wq_sb[:, kc * 256 + ftc * 128:
                                           kc * 256 + (ftc + 1) * 128],
                                rhs=xsl(2 * pi + w, kc, t * 256, (t + 1) * 256),
                                start=(kc == 0), stop=(kc == 1))
                hq = h_p.tile([128, 1024], f16, tag="hq")
                hq_v = hq[:].rearrange("p (w f e) -> p w f e", w=2, f=2, e=256)
                vq_v = vq[:].rearrange("p (w f e) -> p w f e", w=2, f=2, e=256)
                if t == 0:
                    for w in range(2):
                        for f in range(2):
                            nc.scalar.activation(
                                hq_v[:, w, f, :],
                                psq[:, w * 512 + f * 256: w * 512 + (f + 1) * 256],
                                ACT.Identity, bias=bq_sb[:, f:f + 1], scale=1.0)
                else:
                    nc.vector.tensor_tensor(hq[:], psq[:], vq[:], op=ALU.add)
                spkq_eng.tensor_scalar(sq_v[:, 2 * pi:2 * pi + 2, :, t, :], hq_v,
                                       1.0, None, ALU.is_ge)
                if t < T - 1:
                    ltq = st_p.tile([128, 1024], f16, tag="ltq")
                    nc.vector.tensor_scalar(ltq[:], hq[:], 1.0, 0.5,
                                            ALU.is_lt, ALU.mult)
                    nc.vector.tensor_tensor(vq[:], hq[:], ltq[:], op=ALU.mult)
                    # carry must include +0.5*b_q (bias re-enters h each
                    # step; per-partition since q is feature-major)
                    for w in range(2):
                        for f in range(2):
                            nc.vector.tensor_scalar(
                                vq_v[:, w, f, :], vq_v[:, w, f, :],
                                bq_sb[:, f:f + 1], None, ALU.add)

            # collective buffers (two parts: t{0,1,2} and t{3})
            rb_in0 = dram_p.tile([128, 1536], f16)
            rb_out0 = dram_p.tile([4, 128, 1536], f16)
            rb_in1 = dram_p.tile([128, 512], f16)
            rb_out1 = dram_p.tile([4, 128, 512], f16)

            def emit_collective(nc, part):
                rbi = rb_in0 if part == 0 else rb_in1
                rbo = rb_out0 if part == 0 else rb_out1
                lo, sz, base = (0, 1536, 0) if part == 0 else (1536, 512, 6144)
                nc.sync.dma_start(rbi[:], r_loc[:, lo:lo + sz])
                if sim_mode:
                    for rk in range(4):
                        nc.sync.dma_start(rbo[rk], rbi[:])
                else:
                    nc.gpsimd.collective_compute(
                        "AllGather",
                        mybir.AluOpType.bypass,
                        replica_groups=[[0, 1, 2, 3], [4, 5, 6, 7]],
                        ins=[rbi[:].opt()],
                        outs=[rbo[:].opt()],
                    )
                # single merged redistribute DMA (SP SEQ dispatch is ~700ns per
                # DMA and sits on the critical tail)
                nc.sync.dma_start(
                    r_all[:, base: base + 4 * sz].rearrange("p (k s) -> p k s",
                                                            k=4, s=sz),
                    rbo[:].rearrange("k p s -> p k s"))

            for t in range(T):
                skts = []
                for pi in range(NP):
                    skts.append(emit_kv(nc, pi, t))
                    if pi < NP - QDEFER_PAIRS:
                        emit_q(nc, pi, t)
                for pi in range(NP):
                    emit_r(nc, pi, t, skts[pi])
                if t == 2:
                    emit_collective(nc, 0)   # t{0,1,2}: fully hidden under t=3
            emit_collective(nc, 1)           # t{3}: small, covered by deferred q
            for t in range(T):               # t-outer: parallel deferred chains
                for pi in range(NP - QDEFER_PAIRS, NP):
                    emit_q(nc, pi, t)
            ph1.close()

            # ============ phase 2: kv sums, attention, proj =====================
            ph2 = contextlib.ExitStack()
            # gather views: dims (rk, w) select j; kv_w cols are (t4, slab2, e32)
            # so part-1 (t<3) is kv_w[:, 0:192] and part-2 (t=3) kv_w[:, 192:256]
            rA = r_all[:, 0:6144].rearrange("p (k t a w e) -> p k w t a e",
                                            k=4, t=3, a=2, w=8, e=32)
            rB = r_all[:, 6144:8192].rearrange("p (k a w e) -> p k w a e",
                                               k=4, a=2, w=8, e=32)
            # kv sums: routed gather baked per core, guarded by If on core id
            pid = None if sim_mode else nc.partition_id()
            for r in range(NCORES):
                if sim_mode and r != 0:
                    continue
                b_of = r // 4
                wg = r % 4
                with (contextlib.nullcontext() if sim_mode else tc.If(pid == r)):
                    for wl in range(NW):
                        wglob = wg * 8 + wl
                        idxs = [int(j) for j in routing_idx[b_of, wglob]]
                        # attn consumes kv_w in window order: DVE handles the
                        # early windows, the idle gpsimd the tail in parallel
                        eng = nc.gpsimd if wl >= NW - GATHER_POOL_EVERY else nc.vector
                        for dst, rv in ((kv_w[wl][:, 0:192], rA),
                                        (kv_w[wl][:, 192:256], rB)):
                            eng.tensor_copy(dst, rv[:, idxs[0] // 8, idxs[0] % 8])
                            for j in idxs[1:]:
                                eng.tensor_tensor(
                                    dst, dst, rv[:, j // 8, j % 8], op=ALU.add)

            ps_bd = ph2.enter_context(tc.tile_pool(name="psbd", bufs=2, space="PSUM"))
            ps_at = ph2.enter_context(tc.tile_pool(name="psat", bufs=2, space="PSUM"))
            ps_pj = ph2.enter_context(tc.tile_pool(name="pspj", bufs=2, space="PSUM"))
            zrow = constp.tile([1, 128], f16)
            nc.vector.memset(zrow[:], 0.0)
            for wi in range(NW):
                # expand kv_w into block-diagonal [128=(hl,d), 128=(hl,e)]
                # stationaries (one per (slab, t)) so attention contracts all
                # 128 partitions in a single matmul per (slab, t).
                kvbd = skv_p.tile([128, 1024], f16, tag="kvbd")
                for slab in range(2):
                    psb2 = ps_bd.tile([128, 512], f32, tag="psbd", name="psb2")
                    for t in range(T):
                        reg = psb2[:, t * 128:(t + 1) * 128]
                        nc.tensor.matmul(reg, lhsT=zrow[:], rhs=ident[0:1, :],
                                         start=True, stop=False,
                                         skip_group_check=True)
                        for hl in range(4):
                            nc.tensor.matmul(
                                reg[32 * hl:32 * (hl + 1), 32 * hl:32 * (hl + 1)],
                                lhsT=ident[32 * hl:32 * (hl + 1),
                                           32 * hl:32 * (hl + 1)],
                                rhs=kv_w[wi][32 * hl:32 * (hl + 1),
                                             t * 64 + slab * 32:
                                             t * 64 + (slab + 1) * 32],
                                start=False, stop=(hl == 3),
                                tile_position=(32 * hl, 32 * hl),
                                skip_group_check=True,
                            )
                    if slab == 0:
                        nc.scalar.activation(kvbd[:, 0:512], psb2[:], ACT.Copy,
                                             bias=0.0, scale=1.0)
                    else:
                        nc.vector.tensor_copy(kvbd[:, 512:1024], psb2[:])

                attn = attn_p.tile([128, 2 * NTOK], bf16, tag="attn")
                for slab in range(2):
                    psa = ps_at.tile([128, 1024], f32, tag="psat", name="psa")
                    for t in range(T):
                        nc.tensor.matmul(
                            psa[:, t * 256:(t + 1) * 256],
                            lhsT=kvbd[:, slab * 512 + t * 128:
                                      slab * 512 + (t + 1) * 128],
                            rhs=sq_all[:, wi * 2048 + slab * NTOK + t * 256:
                                       wi * 2048 + slab * NTOK + (t + 1) * 256],
                            start=True, stop=True,
                        )
                    dst_at = attn[:, slab * NTOK:(slab + 1) * NTOK]
                    # alternate [128,1024] drains between ACT and DVE
                    if slab == 0:
                        nc.scalar.activation(dst_at, psa[:], ACT.Copy,
                                             bias=0.0, scale=1.0)
                    else:
                        nc.vector.tensor_copy(dst_at, psa[:])

                outsb = out_p.tile([128, 2 * NTOK], bf16, tag="outsb")
                for cft in range(2):
                    for nch in range(2):
                        psp = ps_pj.tile([128, 512], f32, tag="pspj", name="psp")
                        for kc in range(2):
                            nc.tensor.matmul(
                                psp[:],
                                lhsT=wp_sb[:, kc * 256 + cft * 128: kc * 256 + (cft + 1) * 128],
                                rhs=attn[:, kc * NTOK + nch * 512: kc * NTOK + (nch + 1) * 512],
                                start=(kc == 0), stop=(kc == 1),
                            )
                        dst_o = outsb[:, cft * NTOK + nch * 512:
                                      cft * NTOK + (nch + 1) * 512]
                        if nch == 0:
                            nc.scalar.activation(dst_o, psp[:], ACT.Identity,
                                                 bias=bp_sb[:, cft:cft + 1], scale=1.0)
                        else:
                            nc.vector.tensor_scalar(dst_o, psp[:],
                                                    bp_sb[:, cft:cft + 1], None,
                                                    ALU.add)
                for cft in range(2):
                    nc.sync.dma_start(out_d[wi, cft], outsb[:, cft * NTOK:(cft + 1) * NTOK])
            ph2.close()

    _dedup_ldweights(nc, mybir)
    _split_sync_waits(nc, mybir, maxw=1)
    return nc


def _host_prepost(x, w_qkv, b_qkv):
    """Window partition, routing."""
    xw = x.reshape(T, B, WT, GT, WH, GH, WW, GW, C) \
          .transpose(0, 1, 2, 4, 6, 3, 5, 7, 8).reshape(T, B, W, S, C)
    xbar = xw.mean(axis=(0, 3))                      # [B, W, C]
    q_reg = xbar @ w_qkv[:, :C] + b_qkv[:C]
    k_reg = xbar @ w_qkv[:, C:2 * C] + b_qkv[C:2 * C]
    a_r = np.einsum('bwc,bvc->bwv', q_reg, k_reg)
    routing_idx = np.argsort(-a_r, axis=-1)[:, :, :TOPK]   # [B, W, TOPK]
    return xw, routing_idx


def kernel(x, w_qkv, b_qkv, w_proj, b_proj):
    x = np.ascontiguousarray(np.asarray(x, dtype=np.float32))
    w_qkv = np.asarray(w_qkv, dtype=np.float32)
    b_qkv = np.asarray(b_qkv, dtype=np.float32)
    w_proj = np.asarray(w_proj, dtype=np.float32)
    b_proj = np.asarray(b_proj, dtype=np.float32)

    xw, routing_idx = _host_prepost(x, w_qkv, b_qkv)

    key = (routing_idx.tobytes(), SPKKV_ENG, SPKQ_ENG, GATHER_POOL_EVERY)
    if key not in _prog_cache:
        _prog_cache.clear()
        _prog_cache[key] = _build_program(routing_idx)
    nc = _prog_cache[key]

    # weights (shared across cores), all pre-halved for the LIF charge h=(v+x)/2
    wkv_arr = (0.5 * w_qkv[:, C:]).reshape(2, 128, 512).astype(np.float16)
    wq_arr = (0.5 * w_qkv[:, :C]).reshape(2, 128, 256).astype(np.float16)
    bkv_arr = (0.5 * b_qkv[C:]).reshape(1, 512).astype(np.float16)
    bq_arr = (0.5 * b_qkv[:C]).reshape(2, 128).T.astype(np.float32).copy()
    wp = (SCALE * w_proj).reshape(2, 128, 256).astype(ml_dtypes.bfloat16)
    bp = b_proj.reshape(2, 128, 1).astype(np.float32)
    ident = np.eye(128, dtype=np.float16)

    in_maps = []
    for r in range(NCORES):
        b_of, wg = r // 4, r % 4
        xwc = xw[:, b_of, wg * 8:(wg + 1) * 8]              # [T, 8, S, C]
        xl = np.ascontiguousarray(
            xwc.transpose(1, 3, 0, 2).reshape(NW, 2, 128, NTOK)).astype(np.float16)
        in_maps.append({
            "x_in": xl,
            "wkv_in": wkv_arr, "wq_in": wq_arr,
            "bkv_in": bkv_arr, "bq_in": bq_arr,
            "wp_in": wp, "bp_in": bp, "ident_in": ident,
        })

    from concourse.bass_utils import run_bass_kernel_spmd
    res = run_bass_kernel_spmd(nc, in_maps, core_ids=list(range(NCORES)))

    # assemble output
    yw = np.empty((T, B, W, S, C), dtype=np.float32)
    for r in range(NCORES):
        b_of, wg = r // 4, r % 4
        o = np.asarray(res.results[r]["out_d"]).astype(np.float32)  # [NW,2,128,NTOK]
        o = o.reshape(NW, 2, 128, T, S).transpose(0, 3, 4, 1, 2).reshape(NW, T, S, C)
        for wl in range(NW):
            yw[:, b_of, wg * 8 + wl] = o[wl]

    y = yw.reshape(T, B, WT, WH, WW, GT, GH, GW, C) \
          .transpose(0, 1, 2, 5, 3, 6, 4, 7, 8).reshape(T, B, Lt, Lh, Lw, C)
    return y


# revision 89
# speedup vs baseline: 1.8477x; 1.0220x over previous
"""BiLevelRoutingAttention Trainium2 kernel (8-core SPMD).

Sharding: core r handles batch b = r//4 and windows w in [ (r%4)*8, (r%4)*8+8 ).
Routing (region top-k) is computed on host via linearity of the mean.

Design notes (evolution from the 3-pass f16 baseline at 242.7us -> 134.2us):
 - Single-pass fp16 QKV matmuls (~1e-3 rel err, budget 2e-2).
 - x loaded into SBUF once (resident); no phase-2 reload.
 - Phase 1 iterates t OUTER / window-pair INNER: at each step the engines see
   4 independent LIF chains (window pairs), so no engine serializes on the
   t-recurrence. R(wi, t) is issued per step (it only needs step-t spikes).
 - Window-PAIR batching: all LIF elementwise ops process two windows per
   instruction ([128, 2048] / [128,1024] ops) to amortize the ~150-300ns
   per-op dispatch + semaphore overhead that otherwise dominates.
 - gpsimd cannot touch PSUM (BIR verifier), so: kv sh0 carry-add on DVE, kv
   sh1 carry injected into PSUM by a PE identity-matmul accumulate + ACT
   drain; gpsimd gets SBUF-only work (q spikes, kv sh1 state mult).
 - kv spikes via saturated ACT sigmoid(4096*(h-1)) (exactly 0/1 in f16
   outside a ~2e-4 band; same act-table set as Copy => no table reloads);
   t=0 spikes on DVE is_ge while ACT is busy with t0 drains.
 - The R AllGather is split t{0,1,2} / t{3}: the big part fires mid-phase-1
   (fully hidden), the t3 part at the end, covered by the deferred last
   pair's q chain. Redistribute is one merged DMA (SP dispatch ~700ns each).
 - Attention uses block-diagonal kv stationaries built on PE (zero-matmul +
   4 identity quadrant copies into PSUM, drained to SBUF f16): one matmul
   per (window, slab, t) contracts all 128 partitions, 4x fewer PE cycles
   than per-head 32-contract matmuls.
 - attn + proj in bf16 (attn counts <= 65536 exact in f32 psum, ~2e-3 after
   rounding), out DMA'd as bf16.

Spike arithmetic exact: spikes {0,1} f16; R counts <= 256 (f16-exact);
kv sums <= 2048 (f16-exact); attn accumulated in f32 PSUM.

Env knobs (defaults = tuned best): KSPKKV=act, KSPKQ=pool, KGAP=4, KQD=1,
KSPT=1, KT0=act, KHB/KSB/KST pool sizes.
"""

import numpy as np
import ml_dtypes
import os as _os

# ---- problem constants (hardcoded per contract) ----
T, B, Lt, Lh, Lw, C = 4, 2, 8, 32, 32, 256
WT, WH, WW = 2, 4, 4
W = WT * WH * WW            # 32 windows
GT, GH, GW = Lt // WT, Lh // WH, Lw // WW
S = GT * GH * GW            # 256 tokens per window
H, D = 8, C // 8            # 8 heads, 32 dim
TOPK = 8
SCALE = float(D) ** -0.5
NCORES = 8
NW = 8                      # windows per core
NTOK = T * S                # 1024 token-instances per window

SPKKV_ENG = _os.environ.get("KSPKKV", "act")
SPKQ_ENG = _os.environ.get("KSPKQ", "pool")
GATHER_POOL_EVERY = int(_os.environ.get("KGAP", "4"))

_prog_cache = {}


def _dedup_ldweights(nc, mybir):
    """Drop an InstLdweights whose stationary operand is byte-identical to the
    immediately preceding PE Ldweights with only Matmults in between (the PE
    weight slot still holds the same data). Waits/updates are folded into the
    next PE instruction."""
    ndrop = 0
    for bb in nc.main_func.blocks:
        new_list = []
        last_sig = None
        pending = None   # (waits, updates) from a dropped ldw
        for ins in bb.instructions:
            tn = type(ins).__name__
            if ins.engine != mybir.EngineType.PE:
                new_list.append(ins)
                continue
            if pending is not None and tn in ("InstLdweights", "InstMatmult"):
                si = ins.sync_info
                w = list(si.on_wait) if si and si.on_wait else []
                u = list(si.on_update) if si and si.on_update else []
                ins.sync_info = mybir.SyncInfo(on_wait=pending[0] + w,
                                               on_update=pending[1] + u)
                pending = None
            if tn == "InstLdweights":
                try:
                    ap = ins.ins[0]
                    sig = repr(ap)
                except Exception:
                    sig = None
                if sig is not None and sig == last_sig:
                    si = ins.sync_info
                    w = list(si.on_wait) if si and si.on_wait else []
                    u = list(si.on_update) if si and si.on_update else []
                    pending = (w, u)
                    ndrop += 1
                    continue
                last_sig = sig
            elif tn != "InstMatmult":
                last_sig = None
            new_list.append(ins)
        assert pending is None or not (pending[0] or pending[1])
        bb.instructions[:] = new_list
    return ndrop


def _split_sync_waits(nc, mybir, maxw=1):
    """walrus in this container rejects >1 sync wait per instruction; split
    excess waits onto NoOp instructions inserted just before."""
    for bb in nc.main_func.blocks:
        new_list = []
        for ins in bb.instructions:
            si = ins.sync_info
            waits = list(si.on_wait) if si is not None and si.on_wait else []
            if len(waits) > maxw:
                extra = waits[:-maxw]
                keep = waits[-maxw:]
                idx = 0
                while extra:
                    chunk, extra = extra[:maxw], extra[maxw:]
                    nop = mybir.InstNoOp(name=f"{ins.name}-wsplit{idx}", ins=[], outs=[])
                    nop.engine = ins.engine
                    nop.sync_info = mybir.SyncInfo(on_wait=chunk, on_update=[])
                    new_list.append(nop)
                    idx += 1
                ins.sync_info = mybir.SyncInfo(
                    on_wait=keep,
                    on_update=list(si.on_update) if si.on_update else [],
                )
            new_list.append(ins)
        bb.instructions[:] = new_list
    return nc


def _build_program(routing_idx, sim_mode=False):
    """routing_idx: [B, W, TOPK] int array (host-computed). Returns nc.
    sim_mode: no collective / no If-chain (single-core TimelineSim)."""
    import contextlib
    import concourse.bass as bass
    import concourse.mybir as mybir
    import concourse.tile as tile

    f32 = mybir.dt.float32
    f16 = mybir.dt.float16
    bf16 = mybir.dt.bfloat16
    ALU = mybir.AluOpType
    ACT = mybir.ActivationFunctionType

    nc = bass.Bass(num_devices=NCORES)
    spkkv_eng = getattr(nc, {"pool": "gpsimd", "dve": "vector", "act": "vector"}[SPKKV_ENG])
    spkq_eng = getattr(nc, {"pool": "gpsimd", "dve": "vector"}[SPKQ_ENG])

    NP = NW // 2  # window pairs

    # ---- I/O ----
    x_in = nc.dram_tensor("x_in", [NW, 2, 128, NTOK], f16, kind="ExternalInput")
    wkv_in = nc.dram_tensor("wkv_in", [2, 128, 512], f16, kind="ExternalInput")
    wq_in = nc.dram_tensor("wq_in", [2, 128, 256], f16, kind="ExternalInput")
    bkv_in = nc.dram_tensor("bkv_in", [1, 512], f16, kind="ExternalInput")
    bq_in = nc.dram_tensor("bq_in", [128, 2], f32, kind="ExternalInput")
    wp_in = nc.dram_tensor("wp_in", [2, 128, 256], bf16, kind="ExternalInput")
    bp_in = nc.dram_tensor("bp_in", [2, 128, 1], f32, kind="ExternalInput")
    ident_in = nc.dram_tensor("ident_in", [128, 128], f16, kind="ExternalInput")
    out_d = nc.dram_tensor("out_d", [NW, 2, 128, NTOK], bf16, kind="ExternalOutput")

    with tile.TileContext(nc) as tc:
        with (
            tc.tile_pool(name="const", bufs=1) as constp,
            tc.tile_pool(name="hbuf", bufs=4) as h_p,
            tc.tile_pool(name="skv", bufs=6) as skv_p,
            tc.tile_pool(name="state", bufs=2) as st_p,
            tc.tile_pool(name="persist", bufs=1) as pers_p,
            tc.tile_pool(name="attn", bufs=2) as attn_p,
            tc.tile_pool(name="outs", bufs=2) as out_p,
            tc.tile_pool(name="dram", bufs=1, space="DRAM") as dram_p,
        ):
            # ---- constants / weights ----
            wkv_sb = constp.tile([128, 2 * 512], f16)
            wq_sb = constp.tile([128, 2 * 256], f16)
            bkv_sb = constp.tile([1, 512], f16)
            x_all = constp.tile([128, NW * 2 * NTOK], f16)  # cols (wi, kc, tok)
            # order startup DMAs so window-0 compute can begin ASAP
            if _os.environ.get("KXO", "2") == "2":
                # strict first-use order: pair 0 needs BOTH windows' x and
                # wq right away; bias rows follow; wp/bp wait till phase 2
                nc.sync.dma_start(wkv_sb[:, 0:512], wkv_in[0])
                nc.sync.dma_start(x_all[:, 0 * NTOK:1 * NTOK], x_in[0, 0])
                nc.sync.dma_start(wkv_sb[:, 512:1024], wkv_in[1])
                nc.sync.dma_start(x_all[:, 1 * NTOK:2 * NTOK], x_in[0, 1])
                nc.sync.dma_start(x_all[:, 2 * NTOK:3 * NTOK], x_in[1, 0])
                nc.sync.dma_start(x_all[:, 3 * NTOK:4 * NTOK], x_in[1, 1])
                nc.sync.dma_start(bkv_sb[:], bkv_in[:])
            elif _os.environ.get("KXO", "2") == "1":
                for kc in range(2):
                    nc.sync.dma_start(wkv_sb[:, kc * 512:(kc + 1) * 512], wkv_in[kc])
                    nc.sync.dma_start(
                        x_all[:, (0 * 2 + kc) * NTOK:(0 * 2 + kc + 1) * NTOK],
                        x_in[0, kc])
                nc.sync.dma_start(bkv_sb[:], bkv_in[:])
                for kc in range(2):
                    nc.sync.dma_start(
                        x_all[:, (1 * 2 + kc) * NTOK:(1 * 2 + kc + 1) * NTOK],
                        x_in[1, kc])
            else:
                for kc in range(2):
                    nc.sync.dma_start(wkv_sb[:, kc * 512:(kc + 1) * 512], wkv_in[kc])
                nc.sync.dma_start(bkv_sb[:], bkv_in[:])
                for wi in range(2):
                    for kc in range(2):
                        nc.sync.dma_start(
                            x_all[:, (wi * 2 + kc) * NTOK:(wi * 2 + kc + 1) * NTOK],
                            x_in[wi, kc])
            wp_sb = constp.tile([128, 2 * 256], bf16)
            if _os.environ.get("KXO", "2") == "2":
                for kc in range(2):
                    nc.sync.dma_start(wq_sb[:, kc * 256:(kc + 1) * 256], wq_in[kc])
            else:
                for kc in range(2):
                    nc.sync.dma_start(wq_sb[:, kc * 256:(kc + 1) * 256], wq_in[kc])
                    nc.sync.dma_start(wp_sb[:, kc * 256:(kc + 1) * 256], wp_in[kc])
            bq_sb = constp.tile([128, 2], f32)
            nc.sync.dma_start(bq_sb[:], bq_in[:])
            bp_sb = constp.tile([128, 2], f32)
            for ftc in range(2):
                nc.sync.dma_start(bp_sb[:, ftc:ftc + 1], bp_in[ftc])
            ones1 = constp.tile([1, 128], f16)
            nc.vector.memset(ones1[:], 1.0)
            negk = constp.tile([128, 1], f32)
            nc.vector.memset(negk[:], -4096.0)
            ident = constp.tile([128, 128], f16)
            nc.sync.dma_start(ident[:], ident_in[:])
            for wi in range(2, NW):
                for kc in range(2):
                    nc.sync.dma_start(
                        x_all[:, (wi * 2 + kc) * NTOK:(wi * 2 + kc + 1) * NTOK],
                        x_in[wi, kc])
            if _os.environ.get("KXO", "2") == "2":
                for kc in range(2):
                    nc.sync.dma_start(wp_sb[:, kc * 256:(kc + 1) * 256], wp_in[kc])

            def xsl(wi, kc, lo, hi):
                return x_all[:, (wi * 2 + kc) * NTOK + lo:(wi * 2 + kc) * NTOK + hi]

            # persistent across phases
            # r_loc cols: (t4, slab2, wi8, e32) - t-major so the collective
            # can ship t{0,1,2} early and only t{3} after the last step
            r_loc = pers_p.tile([128, 2048], f16)
            # r_all cols: part1 (rk4, t3, slab2, w8, e32) then part2 (rk4, slab2, w8, e32)
            r_all = pers_p.tile([128, 8192], f16)
            kv_w = [pers_p.tile([128, 256], f16, name=f"kvw{i}") for i in range(NW)]
            sq_all = pers_p.tile([128, NW * 2048], f16)    # q spikes (wi, f, t, e)
            vkv_w = [pers_p.tile([128, 2048], f16, name=f"vkv{i}") for i in range(NP)]
            vq_w = [pers_p.tile([128, 1024], f16, name=f"vq{i}") for i in range(NP)]

            # ================= phase 1: kv + q qkv/LIF/R, t-outer, pair ops ====
            ph1 = contextlib.ExitStack()
            ps_a = ph1.enter_context(tc.tile_pool(name="psA", bufs=2, space="PSUM"))
            ps_r = ph1.enter_context(tc.tile_pool(name="psr", bufs=2, space="PSUM"))
            ps_b = ph1.enter_context(tc.tile_pool(name="psB", bufs=1, space="PSUM"))
            QDEFER_PAIRS = int(_os.environ.get("KQD", "1"))
            sq_v = sq_all[:].rearrange("p (w f t e) -> p w f t e", w=NW, f=2, t=4, e=256)

            def emit_kv(nc, pi, t):
                """kv matmuls + LIF for window pair (2pi, 2pi+1) at step t.
                Returns the pair spike tile [128, (sh2, w2, feat512)].

                gpsimd cannot touch PSUM, so: sh0 carry-add on DVE; sh1 carry
                enters PSUM via a PE identity-matmul accumulate and the sum is
                drained to f16 by ACT. gpsimd gets SBUF-only state work."""
                vkv = vkv_w[pi]
                pss = []
                for sh in range(2):
                    st = t * 2 + sh
                    ps = ps_a.tile([128, 1024], f32, tag="psa", name="ps")
                    pss.append(ps)
                    for w in range(2):
                        for kc in range(2):
                            nc.tensor.matmul(
                                ps[:, w * 512:(w + 1) * 512],
                                lhsT=xsl(2 * pi + w, kc, st * 128, (st + 1) * 128),
                                rhs=wkv_sb[:, kc * 512:(kc + 1) * 512],
                                start=(kc == 0), stop=False)
                    # bias matmuls adjacent (ldweights of ones1 dedups)
                    bias_last = (sh == 0) or (t == 0)
                    for w in range(2):
                        nc.tensor.matmul(ps[:, w * 512:(w + 1) * 512],
                                         lhsT=ones1[:], rhs=bkv_sb[:],
                                         start=False, stop=bias_last)
                    if not bias_last:
                        # sh1 carry-add on PE: ps += I @ vkv_sh1
                        for w in range(2):
                            nc.tensor.matmul(
                                ps[:, w * 512:(w + 1) * 512],
                                lhsT=ident[:],
                                rhs=vkv[:, 1024 + w * 512:1024 + (w + 1) * 512],
                                start=False, stop=(w == 1))
                skt = skv_p.tile([128, 2048], f16, tag="skt")
                hsb = h_p.tile([128, 2048], f16, tag="hkv")
                # sh0: DVE add (or DVE copy at t=0 - ACT is saturated early);
                # sh1: ACT drain (carry already accumulated in PSUM by the PE)
                if t == 0:
                    nc.vector.tensor_copy(hsb[:, 0:1024], pss[0][:])
                else:
                    nc.vector.tensor_tensor(hsb[:, 0:1024], pss[0][:],
                                            vkv[:, 0:1024], op=ALU.add)
                nc.scalar.activation(hsb[:, 1024:2048], pss[1][:], ACT.Copy,
                                     bias=0.0, scale=1.0)
                # spike: saturated sigmoid(4096*(h-1)) on ACT is exactly 0/1
                # in f16 outside a ~2e-4-wide band around threshold. Early
                # steps (t<2) run is_ge on DVE instead: ACT is the early
                # bottleneck while DVE still has slack.
                if SPKKV_ENG == "act" and t >= 2:
                    nc.scalar.activation(skt[:], hsb[:], ACT.Sigmoid,
                                         bias=negk[:, 0:1], scale=4096.0)
                else:
                    nc.vector.tensor_scalar(skt[:], hsb[:], 1.0, None, ALU.is_ge)
                if t < T - 1:
                    # state: lt on DVE (both halves); mult sh0 on DVE, sh1 on
                    # gpsimd (all SBUF)
                    lt = st_p.tile([128, 2048], f16, tag="ltkv")
                    for sh in range(2):
                        sl = slice(sh * 1024, (sh + 1) * 1024)
                        nc.vector.tensor_scalar(lt[:, sl], hsb[:, sl], 1.0, 0.5,
                                                ALU.is_lt, ALU.mult)
                        eng = nc.vector if sh == 0 else nc.gpsimd
                        eng.tensor_tensor(vkv[:, sl], hsb[:, sl], lt[:, sl],
                                          op=ALU.mult)
                return skt

            def emit_r(nc, pi, t, skt):
                # one [128,128] psum tile per (pair, t): quadrant (w, slab),
                # drained by a single strided ACT copy into r_loc
                psr = ps_r.tile([128, 128], f32, tag="psr", name="psr")
                for w in range(2):
                    for slab in range(2):
                        reg = psr[:, (w * 2 + slab) * 32:(w * 2 + slab + 1) * 32]
                        for hl in range(4):
                            h = slab * 4 + hl
                            for sh in range(2):
                                base = sh * 1024 + w * 512
                                nc.tensor.matmul(
                                    reg[32 * hl:32 * (hl + 1), :],
                                    lhsT=skt[:, base + h * 32: base + (h + 1) * 32],
                                    rhs=skt[:, base + 256 + h * 32: base + 256 + (h + 1) * 32],
                                    start=(sh == 0), stop=(sh == 1),
                                    tile_position=(0, 32 * hl),
                                )
                psr_v = psr[:].rearrange("p (w s e) -> p w s e", w=2, s=2, e=32)
                dst = r_loc[:].rearrange("p (t s wi e) -> p t wi s e",
                                         t=4, s=2, wi=8, e=32)[:, t, 2 * pi:2 * pi + 2]
                nc.scalar.activation(dst, psr_v, ACT.Copy, bias=0.0, scale=1.0)

            def emit_q(nc, pi, t, deferred=False):
                vq = vq_w[pi]
                psq = ps_b.tile([128, 1024], f32, tag="psb", name="psq")
                for w in range(2):
                    for ftc in range(2):
                        for kc in range(2):
                            nc.tensor.matmul(
                                psq[:, w * 512 + ftc * 256: w * 512 + (ftc + 1) * 256],
                                lhsT=wq_sb[:, kc * 256 + ftc * 128:
                                           kc * 256 + (ftc + 1) * 128],
                                rhs=xsl(2 * pi + w, kc, t * 256, (t + 1) * 256),
                                start=(kc == 0), stop=(kc == 1))
                hq = h_p.tile([128, 1024], f16, tag="hq")
                hq_v = hq[:].rearrange("p (w f e) -> p w f e", w=2, f=2, e=256)
                vq_v = vq[:].rearrange("p (w f e) -> p w f e", w=2, f=2, e=256)
                if t == 0:
                    for w in range(2):
                        for f in range(2):
                            nc.scalar.activation(
                                hq_v[:, w, f, :],
                                psq[:, w * 512 + f * 256: w * 512 + (f + 1) * 256],
                                ACT.Identity, bias=bq_sb[:, f:f + 1], scale=1.0)
                else:
                    nc.vector.tensor_tensor(hq[:], psq[:], vq[:], op=ALU.add)
                if deferred and _os.environ.get("KDQS", "1") == "1":
                    # transition: Pool is saturated by the gather, ACT idles -
                    # use the saturated-sigmoid spike there instead
                    nc.scalar.activation(sq_v[:, 2 * pi:2 * pi + 2, :, t, :],
                                         hq_v, ACT.Sigmoid,
                                         bias=negk[:, 0:1], scale=4096.0)
                else:
                    spkq_eng.tensor_scalar(sq_v[:, 2 * pi:2 * pi + 2, :, t, :],
                                           hq_v, 1.0, None, ALU.is_ge)
                if t < T - 1:
                    ltq = st_p.tile([128, 1024], f16, tag="ltq")
                    nc.vector.tensor_scalar(ltq[:], hq[:], 1.0, 0.5,
                                            ALU.is_lt, ALU.mult)
                    nc.vector.tensor_tensor(vq[:], hq[:], ltq[:], op=ALU.mult)
                    # carry must include +0.5*b_q (bias re-enters h each
                    # step; per-partition since q is feature-major)
                    for w in range(2):
                        for f in range(2):
                            nc.vector.tensor_scalar(
                                vq_v[:, w, f, :], vq_v[:, w, f, :],
                                bq_sb[:, f:f + 1], None, ALU.add)

            # collective buffers (two parts: t{0,1,2} and t{3})
            rb_in0 = dram_p.tile([128, 1536], f16)
            rb_out0 = dram_p.tile([4, 128, 1536], f16)
            rb_in1 = dram_p.tile([128, 512], f16)
            rb_out1 = dram_p.tile([4, 128, 512], f16)

            def emit_collective(nc, part):
                rbi = rb_in0 if part == 0 else rb_in1
                rbo = rb_out0 if part == 0 else rb_out1
                lo, sz, base = (0, 1536, 0) if part == 0 else (1536, 512, 6144)
                nc.sync.dma_start(rbi[:], r_loc[:, lo:lo + sz])
                if sim_mode:
                    for rk in range(4):
                        nc.sync.dma_start(rbo[rk], rbi[:])
                else:
                    nc.gpsimd.collective_compute(
                        "AllGather",
                        mybir.AluOpType.bypass,
                        replica_groups=[[0, 1, 2, 3], [4, 5, 6, 7]],
                        ins=[rbi[:].opt()],
                        outs=[rbo[:].opt()],
                    )
                # single merged redistribute DMA (SP SEQ dispatch is ~700ns per
                # DMA and sits on the critical tail)
                nc.sync.dma_start(
                    r_all[:, base: base + 4 * sz].rearrange("p (k s) -> p k s",
                                                            k=4, s=sz),
                    rbo[:].rearrange("k p s -> p k s"))

            for t in range(T):
                skts = []
                for pi in range(NP):
                    skts.append(emit_kv(nc, pi, t))
                    if pi < NP - QDEFER_PAIRS:
                        emit_q(nc, pi, t)
                for pi in range(NP):
                    emit_r(nc, pi, t, skts[pi])
                if t == 2:
                    emit_collective(nc, 0)   # t{0,1,2}: fully hidden under t=3
            emit_collective(nc, 1)           # t{3}: small, covered by deferred q
            for t in range(T):               # t-outer: parallel deferred chains
                for pi in range(NP - QDEFER_PAIRS, NP):
                    emit_q(nc, pi, t, deferred=True)
            ph1.close()

            # ============ phase 2: kv sums, attention, proj =====================
            ph2 = contextlib.ExitStack()
            # gather views: dims (rk, w) select j; kv_w cols are (t4, slab2, e32)
            # so part-1 (t<3) is kv_w[:, 0:192] and part-2 (t=3) kv_w[:, 192:256]
            rA = r_all[:, 0:6144].rearrange("p (k t a w e) -> p k w t a e",
                                            k=4, t=3, a=2, w=8, e=32)
            rB = r_all[:, 6144:8192].rearrange("p (k a w e) -> p k w a e",
                                               k=4, a=2, w=8, e=32)
            # kv sums: routed gather baked per core, guarded by If on core id
            pid = None if sim_mode else nc.partition_id()
            for r in range(NCORES):
                if sim_mode and r != 0:
                    continue
                b_of = r // 4
                wg = r % 4
                with (contextlib.nullcontext() if sim_mode else tc.If(pid == r)):
                    for wl in range(NW):
                        wglob = wg * 8 + wl
                        idxs = [int(j) for j in routing_idx[b_of, wglob]]
                        # attn consumes kv_w in window order: DVE handles the
                        # early windows, the idle gpsimd the tail in parallel
                        eng = nc.gpsimd if wl >= NW - GATHER_POOL_EVERY else nc.vector
                        for dst, rv in ((kv_w[wl][:, 0:192], rA),
                                        (kv_w[wl][:, 192:256], rB)):
                            eng.tensor_copy(dst, rv[:, idxs[0] // 8, idxs[0] % 8])
                            for j in idxs[1:]:
                                eng.tensor_tensor(
                                    dst, dst, rv[:, j // 8, j % 8], op=ALU.add)

            ps_bd = ph2.enter_context(tc.tile_pool(name="psbd", bufs=2, space="PSUM"))
            ps_at = ph2.enter_context(tc.tile_pool(name="psat", bufs=2, space="PSUM"))
            ps_pj = ph2.enter_context(tc.tile_pool(name="pspj", bufs=2, space="PSUM"))
            zrow = constp.tile([1, 128], f16)
            nc.vector.memset(zrow[:], 0.0)
            for wi in range(NW):
                # expand kv_w into block-diagonal [128=(hl,d), 128=(hl,e)]
                # stationaries (one per (slab, t)) so attention contracts all
                # 128 partitions in a single matmul per (slab, t).
                kvbd = skv_p.tile([128, 1024], f16, tag="kvbd")
                for slab in range(2):
                    psb2 = ps_bd.tile([128, 512], f32, tag="psbd", name="psb2")
                    if _os.environ.get("KBDZ", "1") == "1":
                        nc.tensor.matmul(psb2[:], lhsT=zrow[:],
                                         rhs=wkv_sb[0:1, 0:512],
                                         start=True, stop=False,
                                         skip_group_check=True)
                    for t in range(T):
                        reg = psb2[:, t * 128:(t + 1) * 128]
                        if _os.environ.get("KBDZ", "1") != "1":
                            nc.tensor.matmul(reg, lhsT=zrow[:], rhs=ident[0:1, :],
                                             start=True, stop=False,
                                             skip_group_check=True)
                        for hl in range(4):
                            nc.tensor.matmul(
                                reg[32 * hl:32 * (hl + 1), 32 * hl:32 * (hl + 1)],
                                lhsT=ident[32 * hl:32 * (hl + 1),
                                           32 * hl:32 * (hl + 1)],
                                rhs=kv_w[wi][32 * hl:32 * (hl + 1),
                                             t * 64 + slab * 32:
                                             t * 64 + (slab + 1) * 32],
                                start=False, stop=(hl == 3),
                                tile_position=(32 * hl, 32 * hl),
                                skip_group_check=True,
                            )
                    if slab == 0:
                        nc.scalar.activation(kvbd[:, 0:512], psb2[:], ACT.Copy,
                                             bias=0.0, scale=1.0)
                    else:
                        nc.vector.tensor_copy(kvbd[:, 512:1024], psb2[:])

                attn = attn_p.tile([128, 2 * NTOK], bf16, tag="attn")
                for slab in range(2):
                    psa = ps_at.tile([128, 1024], f32, tag="psat", name="psa")
                    for t in range(T):
                        nc.tensor.matmul(
                            psa[:, t * 256:(t + 1) * 256],
                            lhsT=kvbd[:, slab * 512 + t * 128:
                                      slab * 512 + (t + 1) * 128],
                            rhs=sq_all[:, wi * 2048 + slab * NTOK + t * 256:
                                       wi * 2048 + slab * NTOK + (t + 1) * 256],
                            start=True, stop=True,
                        )
                    dst_at = attn[:, slab * NTOK:(slab + 1) * NTOK]
                    # alternate [128,1024] drains between ACT and DVE
                    if slab == 0:
                        nc.scalar.activation(dst_at, psa[:], ACT.Copy,
                                             bias=0.0, scale=1.0)
                    else:
                        nc.vector.tensor_copy(dst_at, psa[:])

                outsb = out_p.tile([128, 2 * NTOK], bf16, tag="outsb")
                for cft in range(2):
                    for nch in range(2):
                        psp = ps_pj.tile([128, 512], f32, tag="pspj", name="psp")
                        for kc in range(2):
                            nc.tensor.matmul(
                                psp[:],
                                lhsT=wp_sb[:, kc * 256 + cft * 128: kc * 256 + (cft + 1) * 128],
                                rhs=attn[:, kc * NTOK + nch * 512: kc * NTOK + (nch + 1) * 512],
                                start=(kc == 0), stop=(kc == 1),
                            )
                        dst_o = outsb[:, cft * NTOK + nch * 512:
                                      cft * NTOK + (nch + 1) * 512]
                        if nch == 0:
                            nc.scalar.activation(dst_o, psp[:], ACT.Identity,
                                                 bias=bp_sb[:, cft:cft + 1], scale=1.0)
                        else:
                            nc.vector.tensor_scalar(dst_o, psp[:],
                                                    bp_sb[:, cft:cft + 1], None,
                                                    ALU.add)
                if wi >= NW - 2 and _os.environ.get("KOSPL", "0") == "1":
                    # tail windows: DMA halves as soon as each drain lands
                    for cft in range(2):
                        for nch in range(2):
                            nc.sync.dma_start(
                                out_d[wi, cft, :, nch * 512:(nch + 1) * 512],
                                outsb[:, cft * NTOK + nch * 512:
                                      cft * NTOK + (nch + 1) * 512])
                else:
                    for cft in range(2):
                        nc.sync.dma_start(out_d[wi, cft],
                                          outsb[:, cft * NTOK:(cft + 1) * NTOK])
            ph2.close()

    _dedup_ldweights(nc, mybir)
    _split_sync_waits(nc, mybir, maxw=1)
    return nc


def _host_prepost(x, w_qkv, b_qkv):
    """Window partition, routing."""
    xw = x.reshape(T, B, WT, GT, WH, GH, WW, GW, C) \
          .transpose(0, 1, 2, 4, 6, 3, 5, 7, 8).reshape(T, B, W, S, C)
    xbar = xw.mean(axis=(0, 3))                      # [B, W, C]
    q_reg = xbar @ w_qkv[:, :C] + b_qkv[:C]
    k_reg = xbar @ w_qkv[:, C:2 * C] + b_qkv[C:2 * C]
    a_r = np.einsum('bwc,bvc->bwv', q_reg, k_reg)
    routing_idx = np.argsort(-a_r, axis=-1)[:, :, :TOPK]   # [B, W, TOPK]
    return xw, routing_idx


def kernel(x, w_qkv, b_qkv, w_proj, b_proj):
    x = np.ascontiguousarray(np.asarray(x, dtype=np.float32))
    w_qkv = np.asarray(w_qkv, dtype=np.float32)
    b_qkv = np.asarray(b_qkv, dtype=np.float32)
    w_proj = np.asarray(w_proj, dtype=np.float32)
    b_proj = np.asarray(b_proj, dtype=np.float32)

    xw, routing_idx = _host_prepost(x, w_qkv, b_qkv)

    key = (routing_idx.tobytes(), SPKKV_ENG, SPKQ_ENG, GATHER_POOL_EVERY)
    if key not in _prog_cache:
        _prog_cache.clear()
        _prog_cache[key] = _build_program(routing_idx)
    nc = _prog_cache[key]

    # weights (shared across cores), all pre-halved for the LIF charge h=(v+x)/2
    wkv_arr = (0.5 * w_qkv[:, C:]).reshape(2, 128, 512).astype(np.float16)
    wq_arr = (0.5 * w_qkv[:, :C]).reshape(2, 128, 256).astype(np.float16)
    bkv_arr = (0.5 * b_qkv[C:]).reshape(1, 512).astype(np.float16)
    bq_arr = (0.5 * b_qkv[:C]).reshape(2, 128).T.astype(np.float32).copy()
    wp = (SCALE * w_proj).reshape(2, 128, 256).astype(ml_dtypes.bfloat16)
    bp = b_proj.reshape(2, 128, 1).astype(np.float32)
    ident = np.eye(128, dtype=np.float16)

    in_maps = []
    for r in range(NCORES):
        b_of, wg = r // 4, r % 4
        xwc = xw[:, b_of, wg * 8:(wg + 1) * 8]              # [T, 8, S, C]
        xl = np.ascontiguousarray(
            xwc.transpose(1, 3, 0, 2).reshape(NW, 2, 128, NTOK)).astype(np.float16)
        in_maps.append({
            "x_in": xl,
            "wkv_in": wkv_arr, "wq_in": wq_arr,
            "bkv_in": bkv_arr, "bq_in": bq_arr,
            "wp_in": wp, "bp_in": bp, "ident_in": ident,
        })

    from concourse.bass_utils import run_bass_kernel_spmd
    res = run_bass_kernel_spmd(nc, in_maps, core_ids=list(range(NCORES)))

    # assemble output
    yw = np.empty((T, B, W, S, C), dtype=np.float32)
    for r in range(NCORES):
        b_of, wg = r // 4, r % 4
        o = np.asarray(res.results[r]["out_d"]).astype(np.float32)  # [NW,2,128,NTOK]
        o = o.reshape(NW, 2, 128, T, S).transpose(0, 3, 4, 1, 2).reshape(NW, T, S, C)
        for wl in range(NW):
            yw[:, b_of, wg * 8 + wl] = o[wl]

    y = yw.reshape(T, B, WT, WH, WW, GT, GH, GW, C) \
          .transpose(0, 1, 2, 5, 3, 6, 4, 7, 8).reshape(T, B, Lt, Lh, Lw, C)
    return y


# revision 90
# speedup vs baseline: 1.8636x; 1.0086x over previous
"""BiLevelRoutingAttention Trainium2 kernel (8-core SPMD).

Sharding: core r handles batch b = r//4 and windows w in [ (r%4)*8, (r%4)*8+8 ).
Routing (region top-k) is computed on host via linearity of the mean.

Design notes (evolution from the 3-pass f16 baseline at 242.7us -> 134.2us):
 - Single-pass fp16 QKV matmuls (~1e-3 rel err, budget 2e-2).
 - x loaded into SBUF once (resident); no phase-2 reload.
 - Phase 1 iterates t OUTER / window-pair INNER: at each step the engines see
   4 independent LIF chains (window pairs), so no engine serializes on the
   t-recurrence. R(wi, t) is issued per step (it only needs step-t spikes).
 - Window-PAIR batching: all LIF elementwise ops process two windows per
   instruction ([128, 2048] / [128,1024] ops) to amortize the ~150-300ns
   per-op dispatch + semaphore overhead that otherwise dominates.
 - gpsimd cannot touch PSUM (BIR verifier), so: kv sh0 carry-add on DVE, kv
   sh1 carry injected into PSUM by a PE identity-matmul accumulate + ACT
   drain; gpsimd gets SBUF-only work (q spikes, kv sh1 state mult).
 - kv spikes via saturated ACT sigmoid(4096*(h-1)) (exactly 0/1 in f16
   outside a ~2e-4 band; same act-table set as Copy => no table reloads);
   t=0 spikes on DVE is_ge while ACT is busy with t0 drains.
 - The R AllGather is split t{0,1,2} / t{3}: the big part fires mid-phase-1
   (fully hidden), the t3 part at the end, covered by the deferred last
   pair's q chain. Redistribute is one merged DMA (SP dispatch ~700ns each).
 - Attention uses block-diagonal kv stationaries built on PE (zero-matmul +
   4 identity quadrant copies into PSUM, drained to SBUF f16): one matmul
   per (window, slab, t) contracts all 128 partitions, 4x fewer PE cycles
   than per-head 32-contract matmuls.
 - attn + proj in bf16 (attn counts <= 65536 exact in f32 psum, ~2e-3 after
   rounding), out DMA'd as bf16.

Spike arithmetic exact: spikes {0,1} f16; R counts <= 256 (f16-exact);
kv sums <= 2048 (f16-exact); attn accumulated in f32 PSUM.

Env knobs (defaults = tuned best): KSPKKV=act, KSPKQ=pool, KGAP=4, KQD=1,
KSPT=1, KT0=act, KHB/KSB/KST pool sizes.
"""

import numpy as np
import ml_dtypes
import os as _os

# ---- problem constants (hardcoded per contract) ----
T, B, Lt, Lh, Lw, C = 4, 2, 8, 32, 32, 256
WT, WH, WW = 2, 4, 4
W = WT * WH * WW            # 32 windows
GT, GH, GW = Lt // WT, Lh // WH, Lw // WW
S = GT * GH * GW            # 256 tokens per window
H, D = 8, C // 8            # 8 heads, 32 dim
TOPK = 8
SCALE = float(D) ** -0.5
NCORES = 8
NW = 8                      # windows per core
NTOK = T * S                # 1024 token-instances per window

SPKKV_ENG = _os.environ.get("KSPKKV", "act")
SPKQ_ENG = _os.environ.get("KSPKQ", "pool")
GATHER_POOL_EVERY = int(_os.environ.get("KGAP", "4"))

_prog_cache = {}


def _dedup_ldweights(nc, mybir):
    """Drop an InstLdweights whose stationary operand is byte-identical to the
    immediately preceding PE Ldweights with only Matmults in between (the PE
    weight slot still holds the same data). Waits/updates are folded into the
    next PE instruction."""
    ndrop = 0
    for bb in nc.main_func.blocks:
        new_list = []
        last_sig = None
        pending = None   # (waits, updates) from a dropped ldw
        for ins in bb.instructions:
            tn = type(ins).__name__
            if ins.engine != mybir.EngineType.PE:
                new_list.append(ins)
                continue
            if pending is not None and tn in ("InstLdweights", "InstMatmult"):
                si = ins.sync_info
                w = list(si.on_wait) if si and si.on_wait else []
                u = list(si.on_update) if si and si.on_update else []
                ins.sync_info = mybir.SyncInfo(on_wait=pending[0] + w,
                                               on_update=pending[1] + u)
                pending = None
            if tn == "InstLdweights":
                try:
                    ap = ins.ins[0]
                    sig = repr(ap)
                except Exception:
                    sig = None
                if sig is not None and sig == last_sig:
                    si = ins.sync_info
                    w = list(si.on_wait) if si and si.on_wait else []
                    u = list(si.on_update) if si and si.on_update else []
                    pending = (w, u)
                    ndrop += 1
                    continue
                last_sig = sig
            elif tn != "InstMatmult":
                last_sig = None
            new_list.append(ins)
        assert pending is None or not (pending[0] or pending[1])
        bb.instructions[:] = new_list
    return ndrop


def _split_sync_waits(nc, mybir, maxw=1):
    """walrus in this container rejects >1 sync wait per instruction; split
    excess waits onto NoOp instructions inserted just before."""
    for bb in nc.main_func.blocks:
        new_list = []
        for ins in bb.instructions:
            si = ins.sync_info
            waits = list(si.on_wait) if si is not None and si.on_wait else []
            if len(waits) > maxw:
                extra = waits[:-maxw]
                keep = waits[-maxw:]
                idx = 0
                while extra:
                    chunk, extra = extra[:maxw], extra[maxw:]
                    nop = mybir.InstNoOp(name=f"{ins.name}-wsplit{idx}", ins=[], outs=[])
                    nop.engine = ins.engine
                    nop.sync_info = mybir.SyncInfo(on_wait=chunk, on_update=[])
                    new_list.append(nop)
                    idx += 1
                ins.sync_info = mybir.SyncInfo(
                    on_wait=keep,
                    on_update=list(si.on_update) if si.on_update else [],
                )
            new_list.append(ins)
        bb.instructions[:] = new_list
    return nc


def _build_program(routing_idx, sim_mode=False):
    """routing_idx: [B, W, TOPK] int array (host-computed). Returns nc.
    sim_mode: no collective / no If-chain (single-core TimelineSim)."""
    import contextlib
    import concourse.bass as bass
    import concourse.mybir as mybir
    import concourse.tile as tile

    f32 = mybir.dt.float32
    f16 = mybir.dt.float16
    bf16 = mybir.dt.bfloat16
    ALU = mybir.AluOpType
    ACT = mybir.ActivationFunctionType

    nc = bass.Bass(num_devices=NCORES)
    spkkv_eng = getattr(nc, {"pool": "gpsimd", "dve": "vector", "act": "vector"}[SPKKV_ENG])
    spkq_eng = getattr(nc, {"pool": "gpsimd", "dve": "vector"}[SPKQ_ENG])

    NP = NW // 2  # window pairs

    # ---- I/O ----
    x_in = nc.dram_tensor("x_in", [NW, 2, 128, NTOK], f16, kind="ExternalInput")
    wkv_in = nc.dram_tensor("wkv_in", [2, 128, 512], f16, kind="ExternalInput")
    wq_in = nc.dram_tensor("wq_in", [2, 128, 256], f16, kind="ExternalInput")
    bkv_in = nc.dram_tensor("bkv_in", [1, 512], f16, kind="ExternalInput")
    bq_in = nc.dram_tensor("bq_in", [128, 2], f32, kind="ExternalInput")
    wp_in = nc.dram_tensor("wp_in", [2, 128, 256], bf16, kind="ExternalInput")
    bp_in = nc.dram_tensor("bp_in", [2, 128, 1], f32, kind="ExternalInput")
    ident_in = nc.dram_tensor("ident_in", [128, 128], f16, kind="ExternalInput")
    out_d = nc.dram_tensor("out_d", [NW, 2, 128, NTOK], bf16, kind="ExternalOutput")

    with tile.TileContext(nc) as tc:
        with (
            tc.tile_pool(name="const", bufs=1) as constp,
            tc.tile_pool(name="hbuf", bufs=4) as h_p,
            tc.tile_pool(name="skv", bufs=6) as skv_p,
            tc.tile_pool(name="state", bufs=2) as st_p,
            tc.tile_pool(name="persist", bufs=1) as pers_p,
            tc.tile_pool(name="attn", bufs=2) as attn_p,
            tc.tile_pool(name="outs", bufs=2) as out_p,
            tc.tile_pool(name="dram", bufs=1, space="DRAM") as dram_p,
        ):
            # ---- constants / weights ----
            wkv_sb = constp.tile([128, 2 * 512], f16)
            wq_sb = constp.tile([128, 2 * 256], f16)
            bkv_sb = constp.tile([1, 512], f16)
            x_all = constp.tile([128, NW * 2 * NTOK], f16)  # cols (wi, kc, tok)
            # order startup DMAs so window-0 compute can begin ASAP
            if _os.environ.get("KXO", "2") == "2":
                # strict first-use order: pair 0 needs BOTH windows' x and
                # wq right away; bias rows follow; wp/bp wait till phase 2
                nc.sync.dma_start(wkv_sb[:, 0:512], wkv_in[0])
                nc.sync.dma_start(x_all[:, 0 * NTOK:1 * NTOK], x_in[0, 0])
                nc.sync.dma_start(wkv_sb[:, 512:1024], wkv_in[1])
                nc.sync.dma_start(x_all[:, 1 * NTOK:2 * NTOK], x_in[0, 1])
                nc.sync.dma_start(x_all[:, 2 * NTOK:3 * NTOK], x_in[1, 0])
                nc.sync.dma_start(x_all[:, 3 * NTOK:4 * NTOK], x_in[1, 1])
                nc.sync.dma_start(bkv_sb[:], bkv_in[:])
            elif _os.environ.get("KXO", "2") == "1":
                for kc in range(2):
                    nc.sync.dma_start(wkv_sb[:, kc * 512:(kc + 1) * 512], wkv_in[kc])
                    nc.sync.dma_start(
                        x_all[:, (0 * 2 + kc) * NTOK:(0 * 2 + kc + 1) * NTOK],
                        x_in[0, kc])
                nc.sync.dma_start(bkv_sb[:], bkv_in[:])
                for kc in range(2):
                    nc.sync.dma_start(
                        x_all[:, (1 * 2 + kc) * NTOK:(1 * 2 + kc + 1) * NTOK],
                        x_in[1, kc])
            else:
                for kc in range(2):
                    nc.sync.dma_start(wkv_sb[:, kc * 512:(kc + 1) * 512], wkv_in[kc])
                nc.sync.dma_start(bkv_sb[:], bkv_in[:])
                for wi in range(2):
                    for kc in range(2):
                        nc.sync.dma_start(
                            x_all[:, (wi * 2 + kc) * NTOK:(wi * 2 + kc + 1) * NTOK],
                            x_in[wi, kc])
            wp_sb = constp.tile([128, 2 * 256], bf16)
            if _os.environ.get("KXO", "2") == "2":
                for kc in range(2):
                    nc.sync.dma_start(wq_sb[:, kc * 256:(kc + 1) * 256], wq_in[kc])
            else:
                for kc in range(2):
                    nc.sync.dma_start(wq_sb[:, kc * 256:(kc + 1) * 256], wq_in[kc])
                    nc.sync.dma_start(wp_sb[:, kc * 256:(kc + 1) * 256], wp_in[kc])
            bq_sb = constp.tile([128, 2], f32)
            nc.sync.dma_start(bq_sb[:], bq_in[:])
            bp_sb = constp.tile([128, 2], f32)
            if _os.environ.get("KXO", "2") != "2":
                for ftc in range(2):
                    nc.sync.dma_start(bp_sb[:, ftc:ftc + 1], bp_in[ftc])
            ones1 = constp.tile([1, 128], f16)
            nc.vector.memset(ones1[:], 1.0)
            negk = constp.tile([128, 1], f32)
            nc.vector.memset(negk[:], -4096.0)
            ident = constp.tile([128, 128], f16)
            nc.sync.dma_start(ident[:], ident_in[:])
            for wi in range(2, NW):
                for kc in range(2):
                    nc.sync.dma_start(
                        x_all[:, (wi * 2 + kc) * NTOK:(wi * 2 + kc + 1) * NTOK],
                        x_in[wi, kc])
            if _os.environ.get("KXO", "2") == "2":
                for kc in range(2):
                    nc.sync.dma_start(wp_sb[:, kc * 256:(kc + 1) * 256], wp_in[kc])
                for ftc in range(2):
                    nc.sync.dma_start(bp_sb[:, ftc:ftc + 1], bp_in[ftc])

            def xsl(wi, kc, lo, hi):
                return x_all[:, (wi * 2 + kc) * NTOK + lo:(wi * 2 + kc) * NTOK + hi]

            # persistent across phases
            # r_loc cols: (t4, slab2, wi8, e32) - t-major so the collective
            # can ship t{0,1,2} early and only t{3} after the last step
            r_loc = pers_p.tile([128, 2048], f16)
            # r_all cols: part1 (rk4, t3, slab2, w8, e32) then part2 (rk4, slab2, w8, e32)
            r_all = pers_p.tile([128, 8192], f16)
            kv_w = [pers_p.tile([128, 256], f16, name=f"kvw{i}") for i in range(NW)]
            sq_all = pers_p.tile([128, NW * 2048], f16)    # q spikes (wi, f, t, e)
            vkv_w = [pers_p.tile([128, 2048], f16, name=f"vkv{i}") for i in range(NP)]
            vq_w = [pers_p.tile([128, 1024], f16, name=f"vq{i}") for i in range(NP)]

            # ================= phase 1: kv + q qkv/LIF/R, t-outer, pair ops ====
            ph1 = contextlib.ExitStack()
            ps_a = ph1.enter_context(tc.tile_pool(name="psA", bufs=2, space="PSUM"))
            ps_r = ph1.enter_context(tc.tile_pool(name="psr", bufs=2, space="PSUM"))
            ps_b = ph1.enter_context(tc.tile_pool(name="psB", bufs=1, space="PSUM"))
            QDEFER_PAIRS = int(_os.environ.get("KQD", "1"))
            sq_v = sq_all[:].rearrange("p (w f t e) -> p w f t e", w=NW, f=2, t=4, e=256)

            def emit_kv(nc, pi, t):
                """kv matmuls + LIF for window pair (2pi, 2pi+1) at step t.
                Returns the pair spike tile [128, (sh2, w2, feat512)].

                gpsimd cannot touch PSUM, so: sh0 carry-add on DVE; sh1 carry
                enters PSUM via a PE identity-matmul accumulate and the sum is
                drained to f16 by ACT. gpsimd gets SBUF-only state work."""
                vkv = vkv_w[pi]
                pss = []
                for sh in range(2):
                    st = t * 2 + sh
                    ps = ps_a.tile([128, 1024], f32, tag="psa", name="ps")
                    pss.append(ps)
                    for w in range(2):
                        for kc in range(2):
                            nc.tensor.matmul(
                                ps[:, w * 512:(w + 1) * 512],
                                lhsT=xsl(2 * pi + w, kc, st * 128, (st + 1) * 128),
                                rhs=wkv_sb[:, kc * 512:(kc + 1) * 512],
                                start=(kc == 0), stop=False)
                    # bias matmuls adjacent (ldweights of ones1 dedups)
                    bias_last = (sh == 0) or (t == 0)
                    for w in range(2):
                        nc.tensor.matmul(ps[:, w * 512:(w + 1) * 512],
                                         lhsT=ones1[:], rhs=bkv_sb[:],
                                         start=False, stop=bias_last)
                    if not bias_last:
                        # sh1 carry-add on PE: ps += I @ vkv_sh1
                        for w in range(2):
                            nc.tensor.matmul(
                                ps[:, w * 512:(w + 1) * 512],
                                lhsT=ident[:],
                                rhs=vkv[:, 1024 + w * 512:1024 + (w + 1) * 512],
                                start=False, stop=(w == 1))
                skt = skv_p.tile([128, 2048], f16, tag="skt")
                hsb = h_p.tile([128, 2048], f16, tag="hkv")
                # sh0: DVE add (or DVE copy at t=0 - ACT is saturated early);
                # sh1: ACT drain (carry already accumulated in PSUM by the PE)
                if t == 0:
                    nc.vector.tensor_copy(hsb[:, 0:1024], pss[0][:])
                else:
                    nc.vector.tensor_tensor(hsb[:, 0:1024], pss[0][:],
                                            vkv[:, 0:1024], op=ALU.add)
                nc.scalar.activation(hsb[:, 1024:2048], pss[1][:], ACT.Copy,
                                     bias=0.0, scale=1.0)
                # spike: saturated sigmoid(4096*(h-1)) on ACT is exactly 0/1
                # in f16 outside a ~2e-4-wide band around threshold. Early
                # steps (t<2) run is_ge on DVE instead: ACT is the early
                # bottleneck while DVE still has slack.
                if SPKKV_ENG == "act" and t >= 2:
                    nc.scalar.activation(skt[:], hsb[:], ACT.Sigmoid,
                                         bias=negk[:, 0:1], scale=4096.0)
                else:
                    nc.vector.tensor_scalar(skt[:], hsb[:], 1.0, None, ALU.is_ge)
                if t < T - 1:
                    # state: lt on DVE (both halves); mult sh0 on DVE, sh1 on
                    # gpsimd (all SBUF)
                    lt = st_p.tile([128, 2048], f16, tag="ltkv")
                    for sh in range(2):
                        sl = slice(sh * 1024, (sh + 1) * 1024)
                        nc.vector.tensor_scalar(lt[:, sl], hsb[:, sl], 1.0, 0.5,
                                                ALU.is_lt, ALU.mult)
                        eng = nc.vector if sh == 0 else nc.gpsimd
                        eng.tensor_tensor(vkv[:, sl], hsb[:, sl], lt[:, sl],
                                          op=ALU.mult)
                return skt

            def emit_r(nc, pi, t, skt):
                # one [128,128] psum tile per (pair, t): quadrant (w, slab),
                # drained by a single strided ACT copy into r_loc
                psr = ps_r.tile([128, 128], f32, tag="psr", name="psr")
                for w in range(2):
                    for slab in range(2):
                        reg = psr[:, (w * 2 + slab) * 32:(w * 2 + slab + 1) * 32]
                        for hl in range(4):
                            h = slab * 4 + hl
                            for sh in range(2):
                                base = sh * 1024 + w * 512
                                nc.tensor.matmul(
                                    reg[32 * hl:32 * (hl + 1), :],
                                    lhsT=skt[:, base + h * 32: base + (h + 1) * 32],
                                    rhs=skt[:, base + 256 + h * 32: base + 256 + (h + 1) * 32],
                                    start=(sh == 0), stop=(sh == 1),
                                    tile_position=(0, 32 * hl),
                                )
                psr_v = psr[:].rearrange("p (w s e) -> p w s e", w=2, s=2, e=32)
                dst = r_loc[:].rearrange("p (t s wi e) -> p t wi s e",
                                         t=4, s=2, wi=8, e=32)[:, t, 2 * pi:2 * pi + 2]
                nc.scalar.activation(dst, psr_v, ACT.Copy, bias=0.0, scale=1.0)

            def emit_q(nc, pi, t, deferred=False):
                vq = vq_w[pi]
                psq = ps_b.tile([128, 1024], f32, tag="psb", name="psq")
                for w in range(2):
                    for ftc in range(2):
                        for kc in range(2):
                            nc.tensor.matmul(
                                psq[:, w * 512 + ftc * 256: w * 512 + (ftc + 1) * 256],
                                lhsT=wq_sb[:, kc * 256 + ftc * 128:
                                           kc * 256 + (ftc + 1) * 128],
                                rhs=xsl(2 * pi + w, kc, t * 256, (t + 1) * 256),
                                start=(kc == 0), stop=(kc == 1))
                hq = h_p.tile([128, 1024], f16, tag="hq")
                hq_v = hq[:].rearrange("p (w f e) -> p w f e", w=2, f=2, e=256)
                vq_v = vq[:].rearrange("p (w f e) -> p w f e", w=2, f=2, e=256)
                if t == 0:
                    for w in range(2):
                        for f in range(2):
                            nc.scalar.activation(
                                hq_v[:, w, f, :],
                                psq[:, w * 512 + f * 256: w * 512 + (f + 1) * 256],
                                ACT.Identity, bias=bq_sb[:, f:f + 1], scale=1.0)
                else:
                    nc.vector.tensor_tensor(hq[:], psq[:], vq[:], op=ALU.add)
                if deferred and _os.environ.get("KDQS", "1") == "1":
                    # transition: Pool is saturated by the gather, ACT idles -
                    # use the saturated-sigmoid spike there instead
                    nc.scalar.activation(sq_v[:, 2 * pi:2 * pi + 2, :, t, :],
                                         hq_v, ACT.Sigmoid,
                                         bias=negk[:, 0:1], scale=4096.0)
                else:
                    spkq_eng.tensor_scalar(sq_v[:, 2 * pi:2 * pi + 2, :, t, :],
                                           hq_v, 1.0, None, ALU.is_ge)
                if t < T - 1:
                    ltq = st_p.tile([128, 1024], f16, tag="ltq")
                    nc.vector.tensor_scalar(ltq[:], hq[:], 1.0, 0.5,
                                            ALU.is_lt, ALU.mult)
                    nc.vector.tensor_tensor(vq[:], hq[:], ltq[:], op=ALU.mult)
                    # carry must include +0.5*b_q (bias re-enters h each
                    # step; per-partition since q is feature-major)
                    for w in range(2):
                        for f in range(2):
                            nc.vector.tensor_scalar(
                                vq_v[:, w, f, :], vq_v[:, w, f, :],
                                bq_sb[:, f:f + 1], None, ALU.add)

            # collective buffers (two parts: t{0,1,2} and t{3})
            rb_in0 = dram_p.tile([128, 1536], f16)
            rb_out0 = dram_p.tile([4, 128, 1536], f16)
            rb_in1 = dram_p.tile([128, 512], f16)
            rb_out1 = dram_p.tile([4, 128, 512], f16)

            def emit_collective(nc, part):
                rbi = rb_in0 if part == 0 else rb_in1
                rbo = rb_out0 if part == 0 else rb_out1
                lo, sz, base = (0, 1536, 0) if part == 0 else (1536, 512, 6144)
                nc.sync.dma_start(rbi[:], r_loc[:, lo:lo + sz])
                if sim_mode:
                    for rk in range(4):
                        nc.sync.dma_start(rbo[rk], rbi[:])
                else:
                    nc.gpsimd.collective_compute(
                        "AllGather",
                        mybir.AluOpType.bypass,
                        replica_groups=[[0, 1, 2, 3], [4, 5, 6, 7]],
                        ins=[rbi[:].opt()],
                        outs=[rbo[:].opt()],
                    )
                # single merged redistribute DMA (SP SEQ dispatch is ~700ns per
                # DMA and sits on the critical tail)
                nc.sync.dma_start(
                    r_all[:, base: base + 4 * sz].rearrange("p (k s) -> p k s",
                                                            k=4, s=sz),
                    rbo[:].rearrange("k p s -> p k s"))

            for t in range(T):
                skts = []
                for pi in range(NP):
                    skts.append(emit_kv(nc, pi, t))
                    if pi < NP - QDEFER_PAIRS:
                        emit_q(nc, pi, t)
                for pi in range(NP):
                    emit_r(nc, pi, t, skts[pi])
                if t == 2:
                    emit_collective(nc, 0)   # t{0,1,2}: fully hidden under t=3
            emit_collective(nc, 1)           # t{3}: small, covered by deferred q
            for t in range(T):               # t-outer: parallel deferred chains
                for pi in range(NP - QDEFER_PAIRS, NP):
                    emit_q(nc, pi, t, deferred=True)
            ph1.close()

            # ============ phase 2: kv sums, attention, proj =====================
            ph2 = contextlib.ExitStack()
            # gather views: dims (rk, w) select j; kv_w cols are (t4, slab2, e32)
            # so part-1 (t<3) is kv_w[:, 0:192] and part-2 (t=3) kv_w[:, 192:256]
            rA = r_all[:, 0:6144].rearrange("p (k t a w e) -> p k w t a e",
                                            k=4, t=3, a=2, w=8, e=32)
            rB = r_all[:, 6144:8192].rearrange("p (k a w e) -> p k w a e",
                                               k=4, a=2, w=8, e=32)
            # kv sums: routed gather baked per core, guarded by If on core id
            pid = None if sim_mode else nc.partition_id()
            for r in range(NCORES):
                if sim_mode and r != 0:
                    continue
                b_of = r // 4
                wg = r % 4
                with (contextlib.nullcontext() if sim_mode else tc.If(pid == r)):
                    for wl in range(NW):
                        wglob = wg * 8 + wl
                        idxs = [int(j) for j in routing_idx[b_of, wglob]]
                        # attn consumes kv_w in window order: DVE handles the
                        # early windows, the idle gpsimd the tail in parallel
                        eng = nc.gpsimd if wl >= NW - GATHER_POOL_EVERY else nc.vector
                        for dst, rv in ((kv_w[wl][:, 0:192], rA),
                                        (kv_w[wl][:, 192:256], rB)):
                            eng.tensor_copy(dst, rv[:, idxs[0] // 8, idxs[0] % 8])
                            for j in idxs[1:]:
                                eng.tensor_tensor(
                                    dst, dst, rv[:, j // 8, j % 8], op=ALU.add)

            ps_bd = ph2.enter_context(tc.tile_pool(name="psbd", bufs=2, space="PSUM"))
            ps_at = ph2.enter_context(tc.tile_pool(name="psat", bufs=2, space="PSUM"))
            ps_pj = ph2.enter_context(tc.tile_pool(name="pspj", bufs=2, space="PSUM"))
            zrow = constp.tile([1, 128], f16)
            nc.vector.memset(zrow[:], 0.0)
            for wi in range(NW):
                # expand kv_w into block-diagonal [128=(hl,d), 128=(hl,e)]
                # stationaries (one per (slab, t)) so attention contracts all
                # 128 partitions in a single matmul per (slab, t).
                kvbd = skv_p.tile([128, 1024], f16, tag="kvbd")
                for slab in range(2):
                    psb2 = ps_bd.tile([128, 512], f32, tag="psbd", name="psb2")
                    if _os.environ.get("KBDZ", "1") == "1":
                        nc.tensor.matmul(psb2[:], lhsT=zrow[:],
                                         rhs=wkv_sb[0:1, 0:512],
                                         start=True, stop=False,
                                         skip_group_check=True)
                    for t in range(T):
                        reg = psb2[:, t * 128:(t + 1) * 128]
                        if _os.environ.get("KBDZ", "1") != "1":
                            nc.tensor.matmul(reg, lhsT=zrow[:], rhs=ident[0:1, :],
                                             start=True, stop=False,
                                             skip_group_check=True)
                        for hl in range(4):
                            nc.tensor.matmul(
                                reg[32 * hl:32 * (hl + 1), 32 * hl:32 * (hl + 1)],
                                lhsT=ident[32 * hl:32 * (hl + 1),
                                           32 * hl:32 * (hl + 1)],
                                rhs=kv_w[wi][32 * hl:32 * (hl + 1),
                                             t * 64 + slab * 32:
                                             t * 64 + (slab + 1) * 32],
                                start=False, stop=(hl == 3),
                                tile_position=(32 * hl, 32 * hl),
                                skip_group_check=True,
                            )
                    if slab == 0:
                        nc.scalar.activation(kvbd[:, 0:512], psb2[:], ACT.Copy,
                                             bias=0.0, scale=1.0)
                    else:
                        nc.vector.tensor_copy(kvbd[:, 512:1024], psb2[:])

                attn = attn_p.tile([128, 2 * NTOK], bf16, tag="attn")
                for slab in range(2):
                    psa = ps_at.tile([128, 1024], f32, tag="psat", name="psa")
                    for t in range(T):
                        nc.tensor.matmul(
                            psa[:, t * 256:(t + 1) * 256],
                            lhsT=kvbd[:, slab * 512 + t * 128:
                                      slab * 512 + (t + 1) * 128],
                            rhs=sq_all[:, wi * 2048 + slab * NTOK + t * 256:
                                       wi * 2048 + slab * NTOK + (t + 1) * 256],
                            start=True, stop=True,
                        )
                    dst_at = attn[:, slab * NTOK:(slab + 1) * NTOK]
                    # alternate [128,1024] drains between ACT and DVE
                    if slab == 0:
                        nc.scalar.activation(dst_at, psa[:], ACT.Copy,
                                             bias=0.0, scale=1.0)
                    else:
                        nc.vector.tensor_copy(dst_at, psa[:])

                outsb = out_p.tile([128, 2 * NTOK], bf16, tag="outsb")
                for cft in range(2):
                    for nch in range(2):
                        psp = ps_pj.tile([128, 512], f32, tag="pspj", name="psp")
                        for kc in range(2):
                            nc.tensor.matmul(
                                psp[:],
                                lhsT=wp_sb[:, kc * 256 + cft * 128: kc * 256 + (cft + 1) * 128],
                                rhs=attn[:, kc * NTOK + nch * 512: kc * NTOK + (nch + 1) * 512],
                                start=(kc == 0), stop=(kc == 1),
                            )
                        dst_o = outsb[:, cft * NTOK + nch * 512:
                                      cft * NTOK + (nch + 1) * 512]
                        if nch == 0:
                            nc.scalar.activation(dst_o, psp[:], ACT.Identity,
                                                 bias=bp_sb[:, cft:cft + 1], scale=1.0)
                        else:
                            nc.vector.tensor_scalar(dst_o, psp[:],
                                                    bp_sb[:, cft:cft + 1], None,
                                                    ALU.add)
                if wi >= NW - 2 and _os.environ.get("KOSPL", "0") == "1":
                    # tail windows: DMA halves as soon as each drain lands
                    for cft in range(2):
                        for nch in range(2):
                            nc.sync.dma_start(
                                out_d[wi, cft, :, nch * 512:(nch + 1) * 512],
                                outsb[:, cft * NTOK + nch * 512:
                                      cft * NTOK + (nch + 1) * 512])
                else:
                    for cft in range(2):
                        nc.sync.dma_start(out_d[wi, cft],
                                          outsb[:, cft * NTOK:(cft + 1) * NTOK])
            ph2.close()

    _dedup_ldweights(nc, mybir)
    _split_sync_waits(nc, mybir, maxw=1)
    return nc


def _host_prepost(x, w_qkv, b_qkv):
    """Window partition, routing."""
    xw = x.reshape(T, B, WT, GT, WH, GH, WW, GW, C) \
          .transpose(0, 1, 2, 4, 6, 3, 5, 7, 8).reshape(T, B, W, S, C)
    xbar = xw.mean(axis=(0, 3))                      # [B, W, C]
    q_reg = xbar @ w_qkv[:, :C] + b_qkv[:C]
    k_reg = xbar @ w_qkv[:, C:2 * C] + b_qkv[C:2 * C]
    a_r = np.einsum('bwc,bvc->bwv', q_reg, k_reg)
    routing_idx = np.argsort(-a_r, axis=-1)[:, :, :TOPK]   # [B, W, TOPK]
    return xw, routing_idx


def kernel(x, w_qkv, b_qkv, w_proj, b_proj):
    x = np.ascontiguousarray(np.asarray(x, dtype=np.float32))
    w_qkv = np.asarray(w_qkv, dtype=np.float32)
    b_qkv = np.asarray(b_qkv, dtype=np.float32)
    w_proj = np.asarray(w_proj, dtype=np.float32)
    b_proj = np.asarray(b_proj, dtype=np.float32)

    xw, routing_idx = _host_prepost(x, w_qkv, b_qkv)

    key = (routing_idx.tobytes(), SPKKV_ENG, SPKQ_ENG, GATHER_POOL_EVERY)
    if key not in _prog_cache:
        _prog_cache.clear()
        _prog_cache[key] = _build_program(routing_idx)
    nc = _prog_cache[key]

    # weights (shared across cores), all pre-halved for the LIF charge h=(v+x)/2
    wkv_arr = (0.5 * w_qkv[:, C:]).reshape(2, 128, 512).astype(np.float16)
    wq_arr = (0.5 * w_qkv[:, :C]).reshape(2, 128, 256).astype(np.float16)
    bkv_arr = (0.5 * b_qkv[C:]).reshape(1, 512).astype(np.float16)
    bq_arr = (0.5 * b_qkv[:C]).reshape(2, 128).T.astype(np.float32).copy()
    wp = (SCALE * w_proj).reshape(2, 128, 256).astype(ml_dtypes.bfloat16)
    bp = b_proj.reshape(2, 128, 1).astype(np.float32)
    ident = np.eye(128, dtype=np.float16)

    in_maps = []
    for r in range(NCORES):
        b_of, wg = r // 4, r % 4
        xwc = xw[:, b_of, wg * 8:(wg + 1) * 8]              # [T, 8, S, C]
        xl = np.ascontiguousarray(
            xwc.transpose(1, 3, 0, 2).reshape(NW, 2, 128, NTOK)).astype(np.float16)
        in_maps.append({
            "x_in": xl,
            "wkv_in": wkv_arr, "wq_in": wq_arr,
            "bkv_in": bkv_arr, "bq_in": bq_arr,
            "wp_in": wp, "bp_in": bp, "ident_in": ident,
        })

    from concourse.bass_utils import run_bass_kernel_spmd
    res = run_bass_kernel_spmd(nc, in_maps, core_ids=list(range(NCORES)))

    # assemble output
    yw = np.empty((T, B, W, S, C), dtype=np.float32)
    for r in range(NCORES):
        b_of, wg = r // 4, r % 4
        o = np.asarray(res.results[r]["out_d"]).astype(np.float32)  # [NW,2,128,NTOK]
        o = o.reshape(NW, 2, 128, T, S).transpose(0, 3, 4, 1, 2).reshape(NW, T, S, C)
        for wl in range(NW):
            yw[:, b_of, wg * 8 + wl] = o[wl]

    y = yw.reshape(T, B, WT, WH, WW, GT, GH, GW, C) \
          .transpose(0, 1, 2, 5, 3, 6, 4, 7, 8).reshape(T, B, Lt, Lh, Lw, C)
    return y
